# revision 53
# baseline (speedup 1.0000x reference)
"""Trainium2 Bass kernel for a causal AttentionBlock (dense transformer).

Model (reference):
    qkv = x @ Wqkv + bqkv ; 16-head causal attention (no out-proj)
    x2  = x + attn_out
    out = x2 + relu(x2 @ W1 + b1) @ W2 + b2

x: [2, 2048, 1024] fp32. 8 NeuronCores.

Sharding (no collectives): data-parallel over (batch, query-chunk). Core c
handles batch b = c//4 and the balanced causal chunk pair (j, 7-j), j = c%4,
of 8x256-row chunks, giving every core the same 512 query rows. Each core
redundantly projects K/V for its whole batch (uniform SPMD program), computes
attention for its rows, then the MLP for its rows. Host concatenates.

All matmul operands are bf16 (fp32 PSUM accumulate); fp32r triggers HAM power
throttling (util limit 0.5 for >50% of the kernel) and costs 1.5 cyc/row on
LDWEIGHTS. The attention residual add runs in fp32 (xqb operand); x2 is then
kept in bf16 (it only feeds bf16 matmuls and the final residual add).

Everything on-chip runs transposed ([feature, row] layout); x arrives
pre-transposed from the host and the output returns transposed, so the kernel
does zero PE transposes. K stays resident in SBUF (no DRAM spill).

Attention processes 8 kv slots per (head, core): the kv row-blocks are
host-permuted so slots 0-3 serve chunk A and B together (N=512 streams,
slot 3 = A's diagonal), slots 4-7 serve B alone (N=256, slot 7 = B's
diagonal). Per-slot 0/-1e9 gates (per-core data) mask disallowed blocks via
the Exp activation bias; diagonal slots add a constant triangular mask.
Softmax denominators come free via a ones-column appended to V; out-of-range
exp is impossible (scores are bounded) so max-subtraction is skipped.
"""
import os
import sys

sys.path.insert(0, "/opt/trn_rl_repo")

import numpy as np
import ml_dtypes

import bass_rust
import concourse.bass as bass
import concourse.mybir as mybir
import concourse.tile as tile
from concourse.bass_utils import run_bass_kernel_spmd

# ---------------------------------------------------------------- constants
B, T, N = 2, 2048, 1024
H, D = 16, 64
NCORES = 8
CH = 256               # query chunk rows
F32 = mybir.dt.float32
BF16 = mybir.dt.bfloat16
NPBF = ml_dtypes.bfloat16

_prog_cache = {}


# ------------------------------------------------------------- wait legalizer
def _legalize_waits(nc):
    """This walrus build accepts <=1 sync wait on most instructions and 0 on
    fp32/fp32r Matmult (fused self-loading LDW). Move excess waits onto bare
    EventSemaphore instructions inserted before, on the same engine."""
    n_split = 0
    for fn in nc.m.functions:
        for blk in fn.blocks:
            insts = blk.instructions
            out = []
            for inst in insts:
                si = inst.sync_info
                waits = list(si.on_wait) if si is not None else []
                tname = type(inst).__name__
                if tname in ("InstMatmult", "InstMatmultMx"):
                    maxw = 0
                    for arg in inst.ins:
                        dt = getattr(arg, "dtype", None)
                        if dt is not None and mybir.dt.size(dt) == 2:
                            maxw = 1
                            break
                else:
                    maxw = 1
                if len(waits) > maxw:
                    extra = waits[:-maxw] if maxw else waits
                    keep = waits[-maxw:] if maxw else []
                    for k, w in enumerate(extra):
                        ev = mybir.InstEventSemaphore(
                            name=f"{inst.name}-lw{k}", ins=[], outs=[]
                        )
                        ev.engine = inst.engine
                        ev.sync_info = bass_rust.SyncInfo(on_wait=[w], on_update=[])
                        out.append(ev)
                        n_split += 1
                    inst.sync_info = bass_rust.SyncInfo(
                        on_wait=keep, on_update=list(si.on_update)
                    )
                out.append(inst)
            insts[:] = out
    return n_split


# ------------------------------------------------------------------- program
def _build_program():
    nc = bass.Bass("TRN2", debug=False, num_devices=NCORES)

    t_ = {}
    t_["xqt16"] = nc.dram_tensor("xqt16", [N, 2 * CH], BF16,
                                 kind="ExternalInput").ap()
    t_["xbt16"] = nc.dram_tensor("xbt16", [N, T], BF16,
                                 kind="ExternalInput").ap()
    t_["wqkv"] = nc.dram_tensor("wqkv", [8, 128, 3 * N], BF16,
                                kind="ExternalInput").ap()
    t_["w1h"] = nc.dram_tensor("w1h", [32, 128, 8, 128], BF16,
                               kind="ExternalInput").ap()
    t_["w2h"] = nc.dram_tensor("w2h", [8, 128, 32, 128], BF16,
                               kind="ExternalInput").ap()
    for nm, sz in (("bqs", N), ("bk", N), ("bv", N), ("b1", 4 * N),
                   ("b2", N)):
        t_[nm] = nc.dram_tensor(nm, [sz], F32, kind="ExternalInput").ap()
    t_["gta"] = nc.dram_tensor("gta", [128, 3], F32, kind="ExternalInput").ap()
    t_["gtb"] = nc.dram_tensor("gtb", [128, 3], F32, kind="ExternalInput").ap()
    t_["md3"] = nc.dram_tensor("md3", [2, 128, 2 * CH], F32,
                               kind="ExternalInput").ap()
    t_["md7"] = nc.dram_tensor("md7", [128, 2 * CH], F32,
                               kind="ExternalInput").ap()
    t_["out"] = nc.dram_tensor("out", [N, 2 * CH], F32,
                               kind="ExternalOutput").ap()

    with tile.TileContext(nc) as tc:
        _emit(nc, tc, t_)
    return nc


def _emit(nc, tc, t_):
    AF = mybir.ActivationFunctionType
    OP = mybir.AluOpType

    with tc.tile_pool(name="const", bufs=1) as const:
        ones = const.tile([128, D], BF16)
        nc.vector.memset(ones[:], 1.0)
        bias = {}
        const_dmas = []
        for nm, w in (("bqs", 8), ("bk", 8), ("bv", 8), ("b1", 32),
                      ("b2", 8)):
            bias[nm] = const.tile([128, w], F32, name=f"b_{nm}")
            const_dmas.append((bias[nm][:],
                               t_[nm].rearrange("(f p) -> p f", p=128)))
        gta = const.tile([128, 3], F32, name="gta")
        const_dmas.append((gta[:], t_["gta"]))
        gtb = const.tile([128, 3], F32, name="gtb")
        const_dmas.append((gtb[:], t_["gtb"]))
        md3 = const.tile([128, 2, 2 * CH], F32, name="md3")
        const_dmas.append((md3[:], t_["md3"].rearrange("s p q -> p s q")))
        md7 = const.tile([128, 2 * CH], F32, name="md7")
        const_dmas.append((md7[:], t_["md7"]))

        with tc.tile_pool(name="x2t", bufs=8) as px2t:
            x2T = [px2t.tile([128, 2 * CH], BF16, tag="x2t", name=f"x2T{i}")
                   for i in range(8)]
            with tc.tile_pool(name="keep", bufs=1) as keep:
                # v_aug[rt]: [128 kv-rows, head h, [v | 1]]
                v_aug = [keep.tile([128, H, D + 1], BF16, tag=f"va{rt}",
                                   name=f"va{rt}") for rt in range(T // 128)]
                kth = [keep.tile([128, T], BF16, tag=f"kth{f}",
                                 name=f"kth{f}") for f in range(8)]
                qT = [keep.tile([128, 2 * CH], BF16, tag=f"qt{f}",
                                name=f"qT{f}") for f in range(8)]
                # xqb[f] = xq.T + bv (attn-out bias folded into the residual)
                xqb = [keep.tile([128, 2 * CH], BF16, tag=f"xqb{f}",
                                 name=f"xqb{f}") for f in range(8)]

                w1p = _fused_proj_attn(nc, tc, AF, OP, t_, bias, ones, gta,
                                       gtb, md3, md7, v_aug, kth, qT, xqb,
                                       x2T, px2t, const_dmas)
            _phase3(nc, tc, AF, OP, t_, bias, x2T, t_["out"], w1p)


def _fused_proj_attn(nc, tc, AF, OP, t_, bias, ones, gta, gtb, md3, md7,
                     v_aug, kth, qT, xqb, x2T, px2t, const_dmas):
    """Projections + attention, fused so the PE never idles (and never drops
    out of its 2.4GHz p-state): Q and V project densely up front; the
    K-projection of head f+1 is fed as filler matmuls into head f's
    latency-bound attention pipeline.

    Attention per head runs the transposed flow and writes x2T = xq + bv +
    attn (fp32). kv slots 0-3 stream chunk A and B together (N=512, slot 3 =
    A diag with constant tri mask on the A half); slots 4-7 stream B alone
    (N=256, slot 7 = B diag). Per-core gates: multiplicative {0,1} on the A
    half post-exp (DVE), additive {0,-1e9} exp bias for B. AV matmuls and
    the normalize/residual chain are software-pipelined one step behind so
    the in-order PE queue never stalls on DVE/ACT latency."""
    xqt16, xbt16, wqkv = (t_["xqt16"], t_["xbt16"], t_["wqkv"])
    with tc.tile_pool(name="fw", bufs=1) as fw, \
         tc.tile_pool(name="fx", bufs=1) as fx, \
         tc.tile_pool(name="ps2s", bufs=4, space="PSUM") as ps2s, \
         tc.tile_pool(name="ps2o", bufs=2, space="PSUM") as ps2o, \
         tc.tile_pool(name="psk", bufs=2, space="PSUM") as psk:

        # ones column of v_aug (all heads: [v | 1])
        for rt in range(T // 128):
            nc.vector.memset(v_aug[rt][:, :, D:D + 1], 1.0)

        wtk = [fw.tile([128, N], BF16, tag=f"wtk{kc}", name=f"wtk{kc}")
               for kc in range(8)]
        xbT = [fx.tile([128, T], BF16, tag=f"xbt{kc}", name=f"xbT{kc}")
               for kc in range(8)]

        pend = [None]

        def defer(fn):
            if pend[0] is not None:
                pend[0]()
            pend[0] = fn

        with tc.tile_pool(name="fv", bufs=1) as fv:
            wtv = [fv.tile([128, N], BF16, tag=f"wtv{kc}", name=f"wtv{kc}")
                   for kc in range(8)]
            with tc.tile_pool(name="fq", bufs=1) as fq:
                wtq = [fq.tile([128, N], BF16, tag=f"wtq{kc}",
                               name=f"wtq{kc}") for kc in range(8)]
                xqm = [fq.tile([128, 2 * CH], BF16, tag=f"xqm{f}",
                               name=f"xqm{f}") for f in range(8)]
                # first-use tensors split across idle queues for parallel DMA
                for kc in range(8):
                    q = (nc.sync, nc.gpsimd)[kc % 2]
                    q.dma_start(wtq[kc][:], wqkv[kc, :, 0:N])
                for f in range(8):
                    nc.scalar.dma_start(xqm[f][:],
                                        xqt16[f * 128:(f + 1) * 128, :])
                for kc in range(8):
                    nc.sync.dma_start(wtv[kc][:], wqkv[kc, :, 2 * N:3 * N])
                for kc in range(8):
                    nc.sync.dma_start(xbT[kc][:],
                                      xbt16[kc * 128:(kc + 1) * 128, :])
                for kc in range(8):
                    nc.sync.dma_start(wtk[kc][:], wqkv[kc, :, N:2 * N])
                # constants (biases/gates/masks): first use is ~30us in, so
                # they queue behind the bandwidth-critical front tensors
                for dst, src_ap in const_dmas:
                    nc.gpsimd.dma_start(dst, src_ap)

                # --- Q projection: qT[f] = (Wq.T @ xq.T) * 0.125 + bqs
                for f in range(8):
                    pp = psk.tile([128, 2 * CH], F32, tag="kproj")
                    for kc in range(8):
                        nc.tensor.matmul(
                            pp[:], wtq[kc][:, f * 128:(f + 1) * 128],
                            xqm[kc][:], start=(kc == 0), stop=(kc == 7))
                    defer(lambda pp=pp, f=f: nc.scalar.activation(
                        qT[f][:], pp[:], AF.Identity,
                        bias=bias["bqs"][:, f:f + 1], scale=0.125))
                if pend[0] is not None:
                    pend[0]()
                pend[0] = None
                # residual operand, attn-out bias folded in: xqb = xq.T + bv
                for f in range(8):
                    nc.vector.tensor_scalar_add(xqb[f][:], xqm[f][:],
                                                bias["bv"][:, f:f + 1])

            # --- V projection, dense (v_aug complete before attention)
            for rb in range(4):
                for rt in range(4):
                    for nb in range(2):
                        pp = psk.tile([128, 512], F32, tag="kproj")
                        for kc in range(8):
                            nc.tensor.matmul(
                                pp[:],
                                xbT[kc][:, rb * 512 + rt * 128:
                                         rb * 512 + (rt + 1) * 128],
                                wtv[kc][:, nb * 512:(nb + 1) * 512],
                                start=(kc == 0), stop=(kc == 7))
                        defer(lambda pp=pp, rb=rb, rt=rt, nb=nb:
                              nc.vector.tensor_copy(
                                  v_aug[rb * 4 + rt][:, nb * 8:(nb + 1) * 8,
                                                     0:D],
                                  pp[:].rearrange("p (h d) -> p h d", d=D)))
            if pend[0] is not None:
                pend[0]()
            pend[0] = None

        # first W1 tiles: allocated after the q/v weight pools free their
        # space (and outside the keep pool), so their DMAs stream during
        # attention instead of waiting for it to drain at the phase boundary
        w1p = [px2t.tile([128, 8, 128], BF16, tag=f"w1p{i}", bufs=1,
                         name=f"w1p{i}") for i in range(4)]
        for m in range(4):
            nc.sync.dma_start(w1p[m][:], t_["w1h"][m, :, :, :])

        def kproj_fillers(f):
            """One closure per instruction of head f's K projection:
            kth[f][:, rb*512:+512] = (Wk[:, f].T @ xb.T + bk) as bf16."""
            out = []
            for rb in range(4):
                holder = {}
                for kc in range(9):
                    def go(rb=rb, kc=kc, holder=holder, f=f):
                        if kc == 0:
                            holder["pp"] = psk.tile([128, 512], F32,
                                                    tag="kproj",
                                                    name=f"kpp{f}_{rb}")
                        if kc < 8:
                            nc.tensor.matmul(
                                holder["pp"][:],
                                wtk[kc][:, f * 128:(f + 1) * 128],
                                xbT[kc][:, rb * 512:(rb + 1) * 512],
                                start=(kc == 0), stop=(kc == 7))
                        else:
                            nc.vector.tensor_scalar_add(
                                kth[f][:, rb * 512:(rb + 1) * 512],
                                holder["pp"][:], bias["bk"][:, f:f + 1])
                    out.append(go)
            return out

        # K projection of head 0, dense; heads 1-7 become attention filler
        for fn in kproj_fillers(0):
            fn()
        fillers = []

        def fill(n=1):
            for _ in range(n):
                if fillers:
                    fillers.pop(0)()

        _attention(nc, tc, AF, OP, bias, ones, gta, gtb, md3, md7, v_aug,
                   kth, qT, xqb, x2T, ps2s, ps2o, fillers,
                   kproj_fillers, fill)
    return w1p


def _attention(nc, tc, AF, OP, bias, ones, gta, gtb, md3, md7, v_aug,
               kth, qT, xqb, x2T, ps2s, ps2o, fillers,
               kproj_fillers, fill):
    with tc.tile_pool(name="p2m", bufs=1) as p2m, \
         tc.tile_pool(name="p2w", bufs=6) as p2w:

        # odd-head residual operands shifted down to partitions 0:64
        xqlo_t = [p2m.tile([128, 2 * CH], BF16, tag=f"xql{i}",
                           name=f"xq_lo{i}") for i in range(8)]

        def xq_lo(f):
            return xqlo_t[f][0:D, :]

        for f in range(8):
            nc.gpsimd.dma_start(xq_lo(f), xqb[f][D:128, :])

        pending = []          # AV work deferred one slot-iteration
        fin_q = []            # finalize work deferred one half-unit

        def flush():
            while len(pending) > 2:
                pending.pop(0)()
            while len(fin_q) > 1:
                fin_q.pop(0)()

        def start_fin(acc):
            # reciprocal of the [1,512] denominator row would use one DVE
            # lane (6.8ns/elem serial). Shuffle it to [128,4] via tiny gpsimd
            # DMAs, reciprocate on 128 lanes, shuffle back: ~10x less DVE
            # occupancy; the latency hides behind the next unit's pipeline.
            dsb = p2w.tile([128, 2 * CH], F32, tag="dsb", bufs=2)
            nc.vector.tensor_copy(dsb[D:D + 1, :], acc[D:D + 1, :])
            dt = p2w.tile([128, 4], F32, tag="dt", bufs=2)
            nc.gpsimd.dma_start(dt[:], dsb[D:D + 1, :])
            rt = p2w.tile([128, 4], F32, tag="rt", bufs=2)
            nc.vector.reciprocal(rt[:], dt[:])
            rec = p2w.tile([128, 2 * CH], F32, tag="rec", bufs=2)
            nc.gpsimd.dma_start(rec[D:D + 1, :], rt[:])
            recb = p2w.tile([128, 2 * CH], BF16, tag="recb", bufs=2)
            nc.vector.tensor_copy(recb[D:D + 1, :], rec[D:D + 1, :])
            return recb

        for f in range(8):
            if f + 1 < 8:
                fillers.extend(kproj_fillers(f + 1))
            x2lo = p2w.tile([128, 2 * CH], BF16, tag="x2lo", name=f"x2lo{f}",
                            bufs=2)
            for hp in range(2):
                h = 2 * f + hp
                po = D * hp
                qh = qT[f][po:po + D, :]
                acc = ps2o.tile([128, 2 * CH], F32, tag="po")

                # slots 0-3: combined [A|B] streams (N=512)
                for slot in range(4):
                    for s in range(2):
                        c = 2 * slot + s
                        ps = ps2s.tile([128, 2 * CH], F32, tag="ps")
                        nc.tensor.matmul(
                            ps[:], kth[f][po:po + D, c * 128:(c + 1) * 128],
                            qh[:], start=True, stop=True)
                        fill(2)
                        ex = p2w.tile([128, 2 * CH], BF16, tag="ex", bufs=8)
                        if slot == 3:
                            sm = p2w.tile([128, 2 * CH], F32, tag="sm",
                                          bufs=2)
                            nc.vector.tensor_tensor(
                                out=sm[:], in0=ps[:], in1=md3[:, s, :],
                                op=OP.add)
                            nc.scalar.activation(ex[:], sm[:], AF.Exp)
                        else:
                            nc.scalar.activation(ex[:], ps[:], AF.Exp)
                            # zero the A half where this slot's block is
                            # causally disallowed (per-core gate in {0,1})
                            nc.vector.tensor_scalar_mul(
                                ex[:, 0:CH], ex[:, 0:CH],
                                gta[:, slot:slot + 1])
                        flush()

                        def mk_av(ex=ex, c=c, h=h, acc=acc):
                            def go():
                                nc.tensor.matmul(
                                    acc[0:D + 1, :], v_aug[c][:, h, :],
                                    ex[:], start=(c == 0), stop=False,
                                    skip_group_check=True)
                            return go
                        pending.append(mk_av())

                while pending:
                    pending.pop(0)()

                # slots 4-7: B-only streams (one ps/exp per slot, 2x N=256)
                for slot in range(4, 8):
                    ps = ps2s.tile([128, 2 * CH], F32, tag="ps")
                    for s in range(2):
                        c = 2 * slot + s
                        nc.tensor.matmul(
                            ps[:, s * CH:(s + 1) * CH],
                            kth[f][po:po + D, c * 128:(c + 1) * 128],
                            qh[:, CH:2 * CH], start=True, stop=True)
                        fill(1)
                    ex = p2w.tile([128, 2 * CH], BF16, tag="ex", bufs=8)
                    if slot == 7:
                        sm = p2w.tile([128, 2 * CH], F32, tag="sm", bufs=2)
                        nc.vector.tensor_tensor(
                            out=sm[:], in0=ps[:], in1=md7[:], op=OP.add)
                        nc.scalar.activation(ex[:], sm[:], AF.Exp)
                    else:
                        nc.scalar.activation(ex[:], ps[:], AF.Exp,
                                             bias=gtb[:, slot - 4:slot - 3])
                    flush()

                    def mk_av(ex=ex, slot=slot, h=h, acc=acc):
                        def go():
                            for s in range(2):
                                c = 2 * slot + s
                                nc.tensor.matmul(
                                    acc[0:D + 1, CH:2 * CH],
                                    v_aug[c][:, h, :],
                                    ex[:, s * CH:(s + 1) * CH],
                                    start=False, stop=(c == 15),
                                    skip_group_check=True)
                        return go
                    pending.append(mk_av())

                while pending:
                    pending.pop(0)()
                recb = start_fin(acc)
                fin_q.append(_mk_fin(nc, AF, OP, p2w, ps2s, ones, recb,
                                     acc, f, hp, xqb, xq_lo, x2T, x2lo))
            # kth[f+1] must be fully emitted before f+1's QKs reference it
            while fillers:
                fillers.pop(0)()
        for fn in fin_q:
            fn()
        fin_q.clear()


def _mk_fin(nc, AF, OP, p2w, ps2s, ones, recb, acc, f, hp,
            xqb, xq_lo, x2T, x2lo):
    def go():
        pb = ps2s.tile([128, 2 * CH], F32, tag="ps", name=f"pb{f}_{hp}")
        nc.tensor.matmul(pb[0:D, :], ones[D:D + 1, :], recb[D:D + 1, :],
                         start=True, stop=True)
        sb = p2w.tile([128, 2 * CH], F32, tag="sb", bufs=2)
        nc.vector.tensor_copy(sb[0:D, :], pb[0:D, :])
        tt = p2w.tile([128, 2 * CH], F32, tag="tt", bufs=2)
        nc.vector.tensor_tensor(out=tt[0:D, :], in0=acc[0:D, :],
                                in1=sb[0:D, :], op=OP.mult)
        if hp == 0:
            nc.vector.tensor_tensor(
                out=x2T[f][0:D, :], in0=tt[0:D, :],
                in1=xqb[f][0:D, :], op=OP.add)
        else:
            nc.vector.tensor_tensor(
                out=x2lo[0:D, :], in0=tt[0:D, :],
                in1=xq_lo(f), op=OP.add)
            nc.gpsimd.dma_start(x2T[f][D:128, :], x2lo[0:D, :])
    return go


def _phase3(nc, tc, AF, OP, t_, bias, x2T, out, w1p):
    """MLP (transposed) + residual; output stays transposed (host undoes)."""
    w1h, w2h = t_["w1h"], t_["w2h"]
    with tc.tile_pool(name="p3h", bufs=8) as p3h, \
         tc.tile_pool(name="p3w1", bufs=4) as p3w1, \
         tc.tile_pool(name="p3w2", bufs=2) as p3w2, \
         tc.tile_pool(name="p3y", bufs=1) as p3y, \
         tc.tile_pool(name="ps3p", bufs=4, space="PSUM") as ps3p:

        hT = [p3h.tile([128, 4, 2 * CH], BF16, tag="ht", name=f"hT{i}")
              for i in range(8)]
        w1ss = {m: w1p[m] for m in range(4)}

        def load_w1(m):
            w1ss[m] = p3w1.tile([128, 8, 128], BF16, tag="w1s", name=f"w1s{m}")
            nc.sync.dma_start(w1ss[m][:], w1h[m, :, :, :])

        load_w1(4)
        load_w1(5)
        ev_pend = [None]
        for m in range(32):
            if 4 <= m + 2 < 32 and (m + 2) not in w1ss:
                load_w1(m + 2)
            w1s = w1ss.pop(m)
            pp = ps3p.tile([128, 2 * CH], F32, tag="proj")
            for kc in range(8):
                nc.tensor.matmul(pp[:], w1s[:, kc, :], x2T[kc][:],
                                 start=(kc == 0), stop=(kc == 7))
            if ev_pend[0] is not None:
                ev_pend[0]()
            ev_pend[0] = (lambda pp=pp, m=m: nc.scalar.activation(
                hT[m // 4][:, m % 4, :], pp[:], AF.Relu,
                bias=bias["b1"][:, m:m + 1]))
        # hT is read by MLP2 below: drain the deferred evac BEFORE emitting it
        ev_pend[0]()
        ev_pend[0] = None

        w2ss = {}

        def load_w2(mo):
            w2ss[mo] = p3w2.tile([128, 32, 128], BF16, tag="w2s", bufs=3,
                                 name=f"w2s{mo}")
            nc.sync.dma_start(w2ss[mo][:], w2h[mo, :, :, :])

        load_w2(0)
        load_w2(1)
        for mo in range(8):
            if mo + 2 < 8:
                load_w2(mo + 2)
            w2s = w2ss.pop(mo)
            pp = ps3p.tile([128, 2 * CH], F32, tag="proj")
            for kc in range(32):
                nc.tensor.matmul(pp[:], w2s[:, kc, :], hT[kc // 4][:, kc % 4, :],
                                 start=(kc == 0), stop=(kc == 31))
            ys = p3y.tile([128, 2 * CH], F32, tag="yt", bufs=3, name=f"ys{mo}")
            nc.scalar.activation(ys[:], pp[:], AF.Identity,
                                 bias=bias["b2"][:, mo:mo + 1])
            nc.vector.tensor_tensor(out=ys[:], in0=ys[:], in1=x2T[mo][:],
                                    op=OP.add)
            (nc.gpsimd, nc.sync, nc.scalar)[mo % 3].dma_start(
                out[mo * 128:(mo + 1) * 128, :], ys[:])


# --------------------------------------------------------------- host driver
def _install_ntff_hook():
    """The container's antenv stub lacks axon_hooks; provide it so
    run_bass_kernel_spmd(trace=True) can capture NTFF profiles via libaxon."""
    import types

    try:
        import antenv.axon_hooks  # noqa: F401
        return
    except ImportError:
        pass
    holder = {"h": None}
    mod = types.ModuleType("antenv.axon_hooks")
    mod.set_axon_ntff_profile_hook = lambda h: holder.__setitem__("h", h)
    mod.get_axon_ntff_profile_hook = lambda: holder["h"]
    sys.modules["antenv.axon_hooks"] = mod
    import antenv

    antenv.axon_hooks = mod
    if "/root/.axon_site" not in sys.path:
        sys.path.insert(0, "/root/.axon_site")
    from trn_agent_boot.trn_boot import _ntff_profile_via_ctypes

    so = "/opt/axon/libaxon_pjrt.so"
    if os.path.exists(so):
        mod.set_axon_ntff_profile_hook(_ntff_profile_via_ctypes(so))


def _get_program():
    key = "v2-bf16"
    if key not in _prog_cache:
        nc = _build_program()
        _legalize_waits(nc)
        _prog_cache[key] = nc
    return _prog_cache[key]


def _prep_shared(Wqkv, W1, W2, bqkv, b1, b2):
    wqkv = np.ascontiguousarray(
        Wqkv.reshape(8, 128, 3 * N)).astype(NPBF)
    w1h = np.ascontiguousarray(
        W1.reshape(8, 128, 32, 128).transpose(2, 1, 0, 3)).astype(NPBF)
    w2h = np.ascontiguousarray(
        W2.reshape(32, 128, 8, 128).transpose(2, 1, 0, 3)).astype(NPBF)
    return {
        "wqkv": wqkv, "w1h": w1h, "w2h": w2h,
        "bqs": np.ascontiguousarray(bqkv[:N] * 0.125),
        "bk": np.ascontiguousarray(bqkv[N:2 * N]),
        "bv": np.ascontiguousarray(bqkv[2 * N:]),
        "b1": np.ascontiguousarray(b1), "b2": np.ascontiguousarray(b2),
    }


def _core_chunks(c):
    b, j = c // 4, c % 4
    return b, j, 7 - j


def _slot_blocks(j):
    """kv row-block for each of the 8 slots. Slots 0-2: smallest non-diag
    blocks (always allowed for B); slot 3 = A diag (block j); slots 4-6:
    remaining blocks (B-gated); slot 7 = B diag (block 7-j)."""
    rem = sorted(set(range(8)) - {j, 7 - j})
    return rem[:3] + [j] + rem[3:] + [7 - j]


def _make_gates(j):
    order = _slot_blocks(j)
    gta = np.ones((128, 3), np.float32)   # multiplicative, applied post-exp
    gtb = np.zeros((128, 3), np.float32)  # additive, applied pre-exp (bias)
    for i in range(3):
        if order[i] >= j:            # disallowed for A
            gta[:, i] = 0.0
        if order[4 + i] >= 7 - j:    # disallowed for B
            gtb[:, i] = -1e9
    return gta, gtb


def _tri_mask():
    # [s, p, q]: kv row s*128+p vs query q (within the 256-row chunk)
    kv = (np.arange(2)[:, None, None] * 128 + np.arange(128)[None, :, None])
    q = np.arange(CH)[None, None, :]
    return np.where(kv <= q, 0.0, -1e9).astype(np.float32)


_TRI = _tri_mask()                                    # [2, 128, 256]
_MD3 = np.concatenate([_TRI, np.zeros((2, 128, CH), np.float32)],
                      axis=2)                         # [2, 128, 512]
# [128, s*256+q] flat layout for the B-diagonal mask
_MD7 = np.ascontiguousarray(_TRI.transpose(1, 0, 2).reshape(128, 2 * CH))


def kernel(x, Wqkv, bqkv, W1, b1, W2, b2, _trace=False):
    x = np.asarray(x, dtype=np.float32)
    shared = _prep_shared(np.asarray(Wqkv, np.float32),
                          np.asarray(W1, np.float32),
                          np.asarray(W2, np.float32),
                          np.asarray(bqkv, np.float32),
                          np.asarray(b1, np.float32),
                          np.asarray(b2, np.float32))
    in_maps = []
    for c in range(NCORES):
        b, j, jb = _core_chunks(c)
        xq = np.concatenate(
            [x[b, j * CH:(j + 1) * CH], x[b, jb * CH:(jb + 1) * CH]], axis=0)
        xqt = np.ascontiguousarray(xq.T)
        xbp = x[b].reshape(8, CH, N)[_slot_blocks(j)].reshape(T, N)
        xbt = np.ascontiguousarray(xbp.T)
        gta, gtb = _make_gates(j)
        in_maps.append({
            **shared,
            "xqt16": xqt.astype(NPBF),
            "xbt16": xbt.astype(NPBF),
            "gta": gta, "gtb": gtb, "md3": _MD3, "md7": _MD7,
        })

    nc = _get_program()
    if _trace:
        _install_ntff_hook()
    res = run_bass_kernel_spmd(nc, in_maps, list(range(NCORES)), trace=_trace)

    outf = np.empty((B, T, N), dtype=np.float32)
    for c in range(NCORES):
        b, j, jb = _core_chunks(c)
        o = res.results[c]["out"]
        outf[b, j * CH:(j + 1) * CH] = o[:, :CH].T
        outf[b, jb * CH:(jb + 1) * CH] = o[:, CH:].T
    if _trace:
        kernel.last_results = res
    return outf


# revision 54
# speedup vs baseline: 1.1751x; 1.1751x over previous
"""Trainium2 Bass kernel for a causal AttentionBlock (dense transformer).

Model (reference):
    qkv = x @ Wqkv + bqkv ; 16-head causal attention (no out-proj)
    x2  = x + attn_out
    out = x2 + relu(x2 @ W1 + b1) @ W2 + b2

x: [2, 2048, 1024] fp32. 8 NeuronCores.

Sharding (no collectives): data-parallel over (batch, query-chunk). Core c
handles batch b = c//4 and the balanced causal chunk pair (j, 7-j), j = c%4,
of 8x256-row chunks, giving every core the same 512 query rows. Each core
redundantly projects K/V for its whole batch (uniform SPMD program), computes
attention for its rows, then the MLP for its rows. Host concatenates.

All matmul operands are bf16 (fp32 PSUM accumulate); fp32r triggers HAM power
throttling (util limit 0.5 for >50% of the kernel) and costs 1.5 cyc/row on
LDWEIGHTS. The attention residual add runs in fp32 (xqb operand); x2 is then
kept in bf16 (it only feeds bf16 matmuls and the final residual add).

Everything on-chip runs transposed ([feature, row] layout); x arrives
pre-transposed from the host and the output returns transposed, so the kernel
does zero PE transposes. K stays resident in SBUF (no DRAM spill).

Attention processes 8 kv slots per (head, core): the kv row-blocks are
host-permuted so slots 0-3 serve chunk A and B together (N=512 streams,
slot 3 = A's diagonal), slots 4-7 serve B alone (N=256, slot 7 = B's
diagonal). Per-slot 0/-1e9 gates (per-core data) mask disallowed blocks via
the Exp activation bias; diagonal slots add a constant triangular mask.
Softmax denominators come free via a ones-column appended to V; out-of-range
exp is impossible (scores are bounded) so max-subtraction is skipped.
"""
import os
import sys

sys.path.insert(0, "/opt/trn_rl_repo")

import numpy as np
import ml_dtypes

import bass_rust
import concourse.bass as bass
import concourse.mybir as mybir
import concourse.tile as tile
from concourse.bass_utils import run_bass_kernel_spmd

# ---------------------------------------------------------------- constants
B, T, N = 2, 2048, 1024
H, D = 16, 64
NCORES = 8
CH = 256               # query chunk rows
F32 = mybir.dt.float32
BF16 = mybir.dt.bfloat16
NPBF = ml_dtypes.bfloat16

_prog_cache = {}


# ------------------------------------------------------------- wait legalizer
def _legalize_waits(nc):
    """This walrus build accepts <=1 sync wait on most instructions and 0 on
    fp32/fp32r Matmult (fused self-loading LDW). Move excess waits onto bare
    EventSemaphore instructions inserted before, on the same engine."""
    n_split = 0
    for fn in nc.m.functions:
        for blk in fn.blocks:
            insts = blk.instructions
            out = []
            for inst in insts:
                si = inst.sync_info
                waits = list(si.on_wait) if si is not None else []
                tname = type(inst).__name__
                if tname in ("InstMatmult", "InstMatmultMx"):
                    maxw = 0
                    for arg in inst.ins:
                        dt = getattr(arg, "dtype", None)
                        if dt is not None and mybir.dt.size(dt) == 2:
                            maxw = 1
                            break
                else:
                    maxw = 1
                if len(waits) > maxw:
                    extra = waits[:-maxw] if maxw else waits
                    keep = waits[-maxw:] if maxw else []
                    for k, w in enumerate(extra):
                        ev = mybir.InstEventSemaphore(
                            name=f"{inst.name}-lw{k}", ins=[], outs=[]
                        )
                        ev.engine = inst.engine
                        ev.sync_info = bass_rust.SyncInfo(on_wait=[w], on_update=[])
                        out.append(ev)
                        n_split += 1
                    inst.sync_info = bass_rust.SyncInfo(
                        on_wait=keep, on_update=list(si.on_update)
                    )
                out.append(inst)
            insts[:] = out
    return n_split


# ------------------------------------------------------------------- program
def _build_program():
    nc = bass.Bass("TRN2", debug=False, num_devices=NCORES)

    t_ = {}
    t_["xqt16"] = nc.dram_tensor("xqt16", [N, 2 * CH], BF16,
                                 kind="ExternalInput").ap()
    t_["xbt16"] = nc.dram_tensor("xbt16", [N, T], BF16,
                                 kind="ExternalInput").ap()
    t_["wqkv"] = nc.dram_tensor("wqkv", [8, 128, 3 * N], BF16,
                                kind="ExternalInput").ap()
    t_["w1h"] = nc.dram_tensor("w1h", [32, 128, 8, 128], BF16,
                               kind="ExternalInput").ap()
    t_["w2h"] = nc.dram_tensor("w2h", [8, 128, 32, 128], BF16,
                               kind="ExternalInput").ap()
    for nm, sz in (("bqs", N), ("bk", N), ("bv", N), ("b1", 4 * N),
                   ("b2", N)):
        t_[nm] = nc.dram_tensor(nm, [sz], F32, kind="ExternalInput").ap()
    t_["gta"] = nc.dram_tensor("gta", [128, 3], F32, kind="ExternalInput").ap()
    t_["gtb"] = nc.dram_tensor("gtb", [128, 3], F32, kind="ExternalInput").ap()
    t_["md3"] = nc.dram_tensor("md3", [2, 128, 2 * CH], F32,
                               kind="ExternalInput").ap()
    t_["md7"] = nc.dram_tensor("md7", [128, 2 * CH], F32,
                               kind="ExternalInput").ap()
    t_["out"] = nc.dram_tensor("out", [N, 2 * CH], F32,
                               kind="ExternalOutput").ap()

    with tile.TileContext(nc) as tc:
        _emit(nc, tc, t_)
    return nc


def _emit(nc, tc, t_):
    AF = mybir.ActivationFunctionType
    OP = mybir.AluOpType

    with tc.tile_pool(name="const", bufs=1) as const:
        ones = const.tile([128, D], BF16)
        nc.vector.memset(ones[:], 1.0)
        bias = {}
        const_dmas = []
        for nm, w in (("bqs", 8), ("bk", 8), ("bv", 8), ("b1", 32),
                      ("b2", 8)):
            bias[nm] = const.tile([128, w], F32, name=f"b_{nm}")
            const_dmas.append((bias[nm][:],
                               t_[nm].rearrange("(f p) -> p f", p=128)))
        gta = const.tile([128, 3], F32, name="gta")
        const_dmas.append((gta[:], t_["gta"]))
        gtb = const.tile([128, 3], F32, name="gtb")
        const_dmas.append((gtb[:], t_["gtb"]))
        md3 = const.tile([128, 2, 2 * CH], F32, name="md3")
        const_dmas.append((md3[:], t_["md3"].rearrange("s p q -> p s q")))
        md7 = const.tile([128, 2 * CH], F32, name="md7")
        const_dmas.append((md7[:], t_["md7"]))

        with tc.tile_pool(name="x2t", bufs=8) as px2t:
            x2T = [px2t.tile([128, 2 * CH], BF16, tag="x2t", name=f"x2T{i}")
                   for i in range(8)]
            with tc.tile_pool(name="keep", bufs=1) as keep:
                # v_aug[rt]: [128 kv-rows, head h, [v | 1]]
                v_aug = [keep.tile([128, H, D + 1], BF16, tag=f"va{rt}",
                                   name=f"va{rt}") for rt in range(T // 128)]
                kth = [keep.tile([128, T], BF16, tag=f"kth{f}",
                                 name=f"kth{f}") for f in range(8)]
                qT = [keep.tile([128, 2 * CH], BF16, tag=f"qt{f}",
                                name=f"qT{f}") for f in range(8)]
                # xqb[f] = xq.T + bv (attn-out bias folded into the residual)
                xqb = [keep.tile([128, 2 * CH], BF16, tag=f"xqb{f}",
                                 name=f"xqb{f}") for f in range(8)]

                w1p = _fused_proj_attn(nc, tc, AF, OP, t_, bias, ones, gta,
                                       gtb, md3, md7, v_aug, kth, qT, xqb,
                                       x2T, px2t, const_dmas)
            _phase3(nc, tc, AF, OP, t_, bias, x2T, t_["out"], w1p)


def _fused_proj_attn(nc, tc, AF, OP, t_, bias, ones, gta, gtb, md3, md7,
                     v_aug, kth, qT, xqb, x2T, px2t, const_dmas):
    """Projections + attention, fused so the PE never idles (and never drops
    out of its 2.4GHz p-state): Q and V project densely up front; the
    K-projection of head f+1 is fed as filler matmuls into head f's
    latency-bound attention pipeline.

    Attention per head runs the transposed flow and writes x2T = xq + bv +
    attn (fp32). kv slots 0-3 stream chunk A and B together (N=512, slot 3 =
    A diag with constant tri mask on the A half); slots 4-7 stream B alone
    (N=256, slot 7 = B diag). Per-core gates: multiplicative {0,1} on the A
    half post-exp (DVE), additive {0,-1e9} exp bias for B. AV matmuls and
    the normalize/residual chain are software-pipelined one step behind so
    the in-order PE queue never stalls on DVE/ACT latency."""
    xqt16, xbt16, wqkv = (t_["xqt16"], t_["xbt16"], t_["wqkv"])
    with tc.tile_pool(name="fw", bufs=1) as fw, \
         tc.tile_pool(name="fx", bufs=1) as fx, \
         tc.tile_pool(name="ps2s", bufs=4, space="PSUM") as ps2s, \
         tc.tile_pool(name="ps2o", bufs=2, space="PSUM") as ps2o, \
         tc.tile_pool(name="psk", bufs=2, space="PSUM") as psk:

        # ones column of v_aug (all heads: [v | 1])
        for rt in range(T // 128):
            nc.vector.memset(v_aug[rt][:, :, D:D + 1], 1.0)

        wtk = [fw.tile([128, N], BF16, tag=f"wtk{kc}", name=f"wtk{kc}")
               for kc in range(8)]
        xbT = [fx.tile([128, T], BF16, tag=f"xbt{kc}", name=f"xbT{kc}")
               for kc in range(8)]

        pend = [None]

        def defer(fn):
            if pend[0] is not None:
                pend[0]()
            pend[0] = fn

        with tc.tile_pool(name="fv", bufs=1) as fv:
            wtv = [fv.tile([128, N], BF16, tag=f"wtv{kc}", name=f"wtv{kc}")
                   for kc in range(8)]
            with tc.tile_pool(name="fq", bufs=1) as fq:
                wtq = [fq.tile([128, N], BF16, tag=f"wtq{kc}",
                               name=f"wtq{kc}") for kc in range(8)]
                xqm = [fq.tile([128, 2 * CH], BF16, tag=f"xqm{f}",
                               name=f"xqm{f}") for f in range(8)]
                # first-use tensors split across idle queues for parallel DMA
                for kc in range(8):
                    q = (nc.sync, nc.gpsimd)[kc % 2]
                    q.dma_start(wtq[kc][:], wqkv[kc, :, 0:N])
                for f in range(8):
                    nc.scalar.dma_start(xqm[f][:],
                                        xqt16[f * 128:(f + 1) * 128, :])
                for kc in range(8):
                    nc.sync.dma_start(wtv[kc][:], wqkv[kc, :, 2 * N:3 * N])
                for kc in range(8):
                    nc.sync.dma_start(xbT[kc][:],
                                      xbt16[kc * 128:(kc + 1) * 128, :])
                for kc in range(8):
                    nc.sync.dma_start(wtk[kc][:], wqkv[kc, :, N:2 * N])
                # constants (biases/gates/masks): first use is ~30us in, so
                # they queue behind the bandwidth-critical front tensors
                for dst, src_ap in const_dmas:
                    nc.gpsimd.dma_start(dst, src_ap)

                # --- Q projection: qT[f] = (Wq.T @ xq.T) * 0.125 + bqs
                for f in range(8):
                    pp = psk.tile([128, 2 * CH], F32, tag="kproj")
                    for kc in range(8):
                        nc.tensor.matmul(
                            pp[:], wtq[kc][:, f * 128:(f + 1) * 128],
                            xqm[kc][:], start=(kc == 0), stop=(kc == 7))
                    defer(lambda pp=pp, f=f: nc.scalar.activation(
                        qT[f][:], pp[:], AF.Identity,
                        bias=bias["bqs"][:, f:f + 1], scale=0.125))
                if pend[0] is not None:
                    pend[0]()
                pend[0] = None
                # residual operand, attn-out bias folded in: xqb = xq.T + bv
                for f in range(8):
                    nc.vector.tensor_scalar_add(xqb[f][:], xqm[f][:],
                                                bias["bv"][:, f:f + 1])

            # --- V projection, dense (v_aug complete before attention)
            for rb in range(4):
                for rt in range(4):
                    for nb in range(2):
                        pp = psk.tile([128, 512], F32, tag="kproj")
                        for kc in range(8):
                            nc.tensor.matmul(
                                pp[:],
                                xbT[kc][:, rb * 512 + rt * 128:
                                         rb * 512 + (rt + 1) * 128],
                                wtv[kc][:, nb * 512:(nb + 1) * 512],
                                start=(kc == 0), stop=(kc == 7))
                        defer(lambda pp=pp, rb=rb, rt=rt, nb=nb:
                              nc.vector.tensor_copy(
                                  v_aug[rb * 4 + rt][:, nb * 8:(nb + 1) * 8,
                                                     0:D],
                                  pp[:].rearrange("p (h d) -> p h d", d=D)))
            if pend[0] is not None:
                pend[0]()
            pend[0] = None

        # first W1 tiles: allocated after the q/v weight pools free their
        # space (and outside the keep pool), so their DMAs stream during
        # attention instead of waiting for it to drain at the phase boundary
        w1p = [px2t.tile([128, 8, 128], BF16, tag=f"w1p{i}", bufs=1,
                         name=f"w1p{i}") for i in range(4)]
        for m in range(4):
            nc.sync.dma_start(w1p[m][:], t_["w1h"][m, :, :, :])

        def kproj_fillers(f):
            """One closure per instruction of head f's K projection:
            kth[f][:, rb*512:+512] = (Wk[:, f].T @ xb.T + bk) as bf16."""
            out = []
            for rb in range(4):
                holder = {}
                for kc in range(9):
                    def go(rb=rb, kc=kc, holder=holder, f=f):
                        if kc == 0:
                            holder["pp"] = psk.tile([128, 512], F32,
                                                    tag="kproj",
                                                    name=f"kpp{f}_{rb}")
                        if kc < 8:
                            nc.tensor.matmul(
                                holder["pp"][:],
                                wtk[kc][:, f * 128:(f + 1) * 128],
                                xbT[kc][:, rb * 512:(rb + 1) * 512],
                                start=(kc == 0), stop=(kc == 7))
                        else:
                            nc.vector.tensor_scalar_add(
                                kth[f][:, rb * 512:(rb + 1) * 512],
                                holder["pp"][:], bias["bk"][:, f:f + 1])
                    out.append(go)
            return out

        # K projection of head 0, dense; heads 1-7 become attention filler
        for fn in kproj_fillers(0):
            fn()
        fillers = []

        def fill(n=1):
            for _ in range(n):
                if fillers:
                    fillers.pop(0)()

        _attention(nc, tc, AF, OP, bias, ones, gta, gtb, md3, md7, v_aug,
                   kth, qT, xqb, x2T, ps2s, ps2o, fillers,
                   kproj_fillers, fill)
    return w1p


def _attention(nc, tc, AF, OP, bias, ones, gta, gtb, md3, md7, v_aug,
               kth, qT, xqb, x2T, ps2s, ps2o, fillers,
               kproj_fillers, fill):
    with tc.tile_pool(name="p2m", bufs=1) as p2m, \
         tc.tile_pool(name="p2w", bufs=6) as p2w:

        # odd-head residual operands shifted down to partitions 0:64
        xqlo_t = [p2m.tile([128, 2 * CH], BF16, tag=f"xql{i}",
                           name=f"xq_lo{i}") for i in range(8)]

        def xq_lo(f):
            return xqlo_t[f][0:D, :]

        for f in range(8):
            nc.gpsimd.dma_start(xq_lo(f), xqb[f][D:128, :])

        pending = []          # AV work deferred one slot-iteration
        fin_q = []            # finalize work deferred one half-unit

        def flush():
            while len(pending) > 2:
                pending.pop(0)()
            while len(fin_q) > 1:
                fin_q.pop(0)()

        def start_fin(acc):
            # reciprocal of the [1,512] denominator row would use one DVE
            # lane (6.8ns/elem serial). Shuffle it to [128,4] via tiny gpsimd
            # DMAs, reciprocate on 128 lanes, shuffle back: ~10x less DVE
            # occupancy; the latency hides behind the next unit's pipeline.
            dsb = p2w.tile([128, 2 * CH], F32, tag="dsb", bufs=2)
            nc.vector.tensor_copy(dsb[D:D + 1, :], acc[D:D + 1, :])
            dt = p2w.tile([128, 4], F32, tag="dt", bufs=2)
            nc.gpsimd.dma_start(dt[:], dsb[D:D + 1, :])
            rt = p2w.tile([128, 4], F32, tag="rt", bufs=2)
            nc.vector.reciprocal(rt[:], dt[:])
            rec = p2w.tile([128, 2 * CH], F32, tag="rec", bufs=2)
            nc.gpsimd.dma_start(rec[D:D + 1, :], rt[:])
            recb = p2w.tile([128, 2 * CH], BF16, tag="recb", bufs=2)
            nc.vector.tensor_copy(recb[D:D + 1, :], rec[D:D + 1, :])
            return recb

        for f in range(8):
            if f + 1 < 8:
                fillers.extend(kproj_fillers(f + 1))
            x2lo = p2w.tile([128, 2 * CH], BF16, tag="x2lo", name=f"x2lo{f}",
                            bufs=2)
            # last head: odd half (whose fin ends in a gpsimd DMA hop) goes
            # first so the final x2T write is the direct DVE-add path
            for hp in ((1, 0) if f == 7 else (0, 1)):
                h = 2 * f + hp
                po = D * hp
                qh = qT[f][po:po + D, :]
                acc = ps2o.tile([128, 2 * CH], F32, tag="po")

                # slots 0-3: combined [A|B] streams (N=512)
                for slot in range(4):
                    for s in range(2):
                        c = 2 * slot + s
                        ps = ps2s.tile([128, 2 * CH], F32, tag="ps")
                        nc.tensor.matmul(
                            ps[:], kth[f][po:po + D, c * 128:(c + 1) * 128],
                            qh[:], start=True, stop=True)
                        fill(2)
                        ex = p2w.tile([128, 2 * CH], BF16, tag="ex", bufs=8)
                        if slot == 3:
                            sm = p2w.tile([128, 2 * CH], F32, tag="sm",
                                          bufs=2)
                            nc.vector.tensor_tensor(
                                out=sm[:], in0=ps[:], in1=md3[:, s, :],
                                op=OP.add)
                            nc.scalar.activation(ex[:], sm[:], AF.Exp)
                        else:
                            nc.scalar.activation(ex[:], ps[:], AF.Exp)
                            # zero the A half where this slot's block is
                            # causally disallowed (per-core gate in {0,1})
                            nc.vector.tensor_scalar_mul(
                                ex[:, 0:CH], ex[:, 0:CH],
                                gta[:, slot:slot + 1])
                        flush()

                        def mk_av(ex=ex, c=c, h=h, acc=acc):
                            def go():
                                nc.tensor.matmul(
                                    acc[0:D + 1, :], v_aug[c][:, h, :],
                                    ex[:], start=(c == 0), stop=False,
                                    skip_group_check=True)
                            return go
                        pending.append(mk_av())

                while pending:
                    pending.pop(0)()

                # slots 4-7: B-only streams (one ps/exp per slot, 2x N=256)
                for slot in range(4, 8):
                    ps = ps2s.tile([128, 2 * CH], F32, tag="ps")
                    for s in range(2):
                        c = 2 * slot + s
                        nc.tensor.matmul(
                            ps[:, s * CH:(s + 1) * CH],
                            kth[f][po:po + D, c * 128:(c + 1) * 128],
                            qh[:, CH:2 * CH], start=True, stop=True)
                        fill(1)
                    ex = p2w.tile([128, 2 * CH], BF16, tag="ex", bufs=8)
                    if slot == 7:
                        sm = p2w.tile([128, 2 * CH], F32, tag="sm", bufs=2)
                        nc.vector.tensor_tensor(
                            out=sm[:], in0=ps[:], in1=md7[:], op=OP.add)
                        nc.scalar.activation(ex[:], sm[:], AF.Exp)
                    else:
                        nc.scalar.activation(ex[:], ps[:], AF.Exp,
                                             bias=gtb[:, slot - 4:slot - 3])
                    flush()

                    def mk_av(ex=ex, slot=slot, h=h, acc=acc):
                        def go():
                            for s in range(2):
                                c = 2 * slot + s
                                nc.tensor.matmul(
                                    acc[0:D + 1, CH:2 * CH],
                                    v_aug[c][:, h, :],
                                    ex[:, s * CH:(s + 1) * CH],
                                    start=False, stop=(c == 15),
                                    skip_group_check=True)
                        return go
                    pending.append(mk_av())

                while pending:
                    pending.pop(0)()
                recb = start_fin(acc)
                fin_q.append(_mk_fin(nc, AF, OP, p2w, ps2s, ones, recb,
                                     acc, f, hp, xqb, xq_lo, x2T, x2lo))
            # kth[f+1] must be fully emitted before f+1's QKs reference it
            while fillers:
                fillers.pop(0)()
        for fn in fin_q:
            fn()
        fin_q.clear()


def _mk_fin(nc, AF, OP, p2w, ps2s, ones, recb, acc, f, hp,
            xqb, xq_lo, x2T, x2lo):
    def go():
        pb = ps2s.tile([128, 2 * CH], F32, tag="ps", name=f"pb{f}_{hp}")
        nc.tensor.matmul(pb[0:D, :], ones[D:D + 1, :], recb[D:D + 1, :],
                         start=True, stop=True)
        sb = p2w.tile([128, 2 * CH], F32, tag="sb", bufs=2)
        nc.vector.tensor_copy(sb[0:D, :], pb[0:D, :])
        tt = p2w.tile([128, 2 * CH], F32, tag="tt", bufs=2)
        nc.vector.tensor_tensor(out=tt[0:D, :], in0=acc[0:D, :],
                                in1=sb[0:D, :], op=OP.mult)
        if hp == 0:
            nc.vector.tensor_tensor(
                out=x2T[f][0:D, :], in0=tt[0:D, :],
                in1=xqb[f][0:D, :], op=OP.add)
        else:
            nc.vector.tensor_tensor(
                out=x2lo[0:D, :], in0=tt[0:D, :],
                in1=xq_lo(f), op=OP.add)
            nc.gpsimd.dma_start(x2T[f][D:128, :], x2lo[0:D, :])
    return go


def _phase3(nc, tc, AF, OP, t_, bias, x2T, out, w1p):
    """MLP (transposed) + residual; output stays transposed (host undoes)."""
    w1h, w2h = t_["w1h"], t_["w2h"]
    with tc.tile_pool(name="p3h", bufs=8) as p3h, \
         tc.tile_pool(name="p3w1", bufs=4) as p3w1, \
         tc.tile_pool(name="p3w2", bufs=2) as p3w2, \
         tc.tile_pool(name="p3y", bufs=1) as p3y, \
         tc.tile_pool(name="ps3p", bufs=4, space="PSUM") as ps3p:

        hT = [p3h.tile([128, 4, 2 * CH], BF16, tag="ht", name=f"hT{i}")
              for i in range(8)]
        w1ss = {m: w1p[m] for m in range(4)}

        def load_w1(m):
            w1ss[m] = p3w1.tile([128, 8, 128], BF16, tag="w1s", name=f"w1s{m}")
            nc.sync.dma_start(w1ss[m][:], w1h[m, :, :, :])

        load_w1(4)
        load_w1(5)
        ev_pend = [None]
        for m in range(32):
            if 4 <= m + 2 < 32 and (m + 2) not in w1ss:
                load_w1(m + 2)
            w1s = w1ss.pop(m)
            pp = ps3p.tile([128, 2 * CH], F32, tag="proj")
            for kc in range(8):
                nc.tensor.matmul(pp[:], w1s[:, kc, :], x2T[kc][:],
                                 start=(kc == 0), stop=(kc == 7))
            if ev_pend[0] is not None:
                ev_pend[0]()
            ev_pend[0] = (lambda pp=pp, m=m: nc.scalar.activation(
                hT[m // 4][:, m % 4, :], pp[:], AF.Relu,
                bias=bias["b1"][:, m:m + 1]))
        # hT is read by MLP2 below: drain the deferred evac BEFORE emitting it
        ev_pend[0]()
        ev_pend[0] = None

        w2ss = {}

        def load_w2(mo):
            w2ss[mo] = p3w2.tile([128, 32, 128], BF16, tag="w2s", bufs=3,
                                 name=f"w2s{mo}")
            nc.sync.dma_start(w2ss[mo][:], w2h[mo, :, :, :])

        load_w2(0)
        load_w2(1)
        for mo in range(8):
            if mo + 2 < 8:
                load_w2(mo + 2)
            w2s = w2ss.pop(mo)
            pp = ps3p.tile([128, 2 * CH], F32, tag="proj")
            for kc in range(32):
                nc.tensor.matmul(pp[:], w2s[:, kc, :], hT[kc // 4][:, kc % 4, :],
                                 start=(kc == 0), stop=(kc == 31))
            ys = p3y.tile([128, 2 * CH], F32, tag="yt", bufs=3, name=f"ys{mo}")
            nc.scalar.activation(ys[:], pp[:], AF.Identity,
                                 bias=bias["b2"][:, mo:mo + 1])
            nc.vector.tensor_tensor(out=ys[:], in0=ys[:], in1=x2T[mo][:],
                                    op=OP.add)
            (nc.gpsimd, nc.sync, nc.scalar)[mo % 3].dma_start(
                out[mo * 128:(mo + 1) * 128, :], ys[:])


# --------------------------------------------------------------- host driver
def _install_ntff_hook():
    """The container's antenv stub lacks axon_hooks; provide it so
    run_bass_kernel_spmd(trace=True) can capture NTFF profiles via libaxon."""
    import types

    try:
        import antenv.axon_hooks  # noqa: F401
        return
    except ImportError:
        pass
    holder = {"h": None}
    mod = types.ModuleType("antenv.axon_hooks")
    mod.set_axon_ntff_profile_hook = lambda h: holder.__setitem__("h", h)
    mod.get_axon_ntff_profile_hook = lambda: holder["h"]
    sys.modules["antenv.axon_hooks"] = mod
    import antenv

    antenv.axon_hooks = mod
    if "/root/.axon_site" not in sys.path:
        sys.path.insert(0, "/root/.axon_site")
    from trn_agent_boot.trn_boot import _ntff_profile_via_ctypes

    so = "/opt/axon/libaxon_pjrt.so"
    if os.path.exists(so):
        mod.set_axon_ntff_profile_hook(_ntff_profile_via_ctypes(so))


def _get_program():
    key = "v2-bf16"
    if key not in _prog_cache:
        nc = _build_program()
        _legalize_waits(nc)
        _prog_cache[key] = nc
    return _prog_cache[key]


def _prep_shared(Wqkv, W1, W2, bqkv, b1, b2):
    wqkv = np.ascontiguousarray(
        Wqkv.reshape(8, 128, 3 * N)).astype(NPBF)
    w1h = np.ascontiguousarray(
        W1.reshape(8, 128, 32, 128).transpose(2, 1, 0, 3)).astype(NPBF)
    w2h = np.ascontiguousarray(
        W2.reshape(32, 128, 8, 128).transpose(2, 1, 0, 3)).astype(NPBF)
    return {
        "wqkv": wqkv, "w1h": w1h, "w2h": w2h,
        "bqs": np.ascontiguousarray(bqkv[:N] * 0.125),
        "bk": np.ascontiguousarray(bqkv[N:2 * N]),
        "bv": np.ascontiguousarray(bqkv[2 * N:]),
        "b1": np.ascontiguousarray(b1), "b2": np.ascontiguousarray(b2),
    }


def _core_chunks(c):
    b, j = c // 4, c % 4
    return b, j, 7 - j


def _slot_blocks(j):
    """kv row-block for each of the 8 slots. Slots 0-2: smallest non-diag
    blocks (always allowed for B); slot 3 = A diag (block j); slots 4-6:
    remaining blocks (B-gated); slot 7 = B diag (block 7-j)."""
    rem = sorted(set(range(8)) - {j, 7 - j})
    return rem[:3] + [j] + rem[3:] + [7 - j]


def _make_gates(j):
    order = _slot_blocks(j)
    gta = np.ones((128, 3), np.float32)   # multiplicative, applied post-exp
    gtb = np.zeros((128, 3), np.float32)  # additive, applied pre-exp (bias)
    for i in range(3):
        if order[i] >= j:            # disallowed for A
            gta[:, i] = 0.0
        if order[4 + i] >= 7 - j:    # disallowed for B
            gtb[:, i] = -1e9
    return gta, gtb


def _tri_mask():
    # [s, p, q]: kv row s*128+p vs query q (within the 256-row chunk)
    kv = (np.arange(2)[:, None, None] * 128 + np.arange(128)[None, :, None])
    q = np.arange(CH)[None, None, :]
    return np.where(kv <= q, 0.0, -1e9).astype(np.float32)


_TRI = _tri_mask()                                    # [2, 128, 256]
_MD3 = np.concatenate([_TRI, np.zeros((2, 128, CH), np.float32)],
                      axis=2)                         # [2, 128, 512]
# [128, s*256+q] flat layout for the B-diagonal mask
_MD7 = np.ascontiguousarray(_TRI.transpose(1, 0, 2).reshape(128, 2 * CH))


def kernel(x, Wqkv, bqkv, W1, b1, W2, b2, _trace=False):
    x = np.asarray(x, dtype=np.float32)
    shared = _prep_shared(np.asarray(Wqkv, np.float32),
                          np.asarray(W1, np.float32),
                          np.asarray(W2, np.float32),
                          np.asarray(bqkv, np.float32),
                          np.asarray(b1, np.float32),
                          np.asarray(b2, np.float32))
    in_maps = []
    for c in range(NCORES):
        b, j, jb = _core_chunks(c)
        xq = np.concatenate(
            [x[b, j * CH:(j + 1) * CH], x[b, jb * CH:(jb + 1) * CH]], axis=0)
        xqt = np.ascontiguousarray(xq.T)
        xbp = x[b].reshape(8, CH, N)[_slot_blocks(j)].reshape(T, N)
        xbt = np.ascontiguousarray(xbp.T)
        gta, gtb = _make_gates(j)
        in_maps.append({
            **shared,
            "xqt16": xqt.astype(NPBF),
            "xbt16": xbt.astype(NPBF),
            "gta": gta, "gtb": gtb, "md3": _MD3, "md7": _MD7,
        })

    nc = _get_program()
    if _trace:
        _install_ntff_hook()
    res = run_bass_kernel_spmd(nc, in_maps, list(range(NCORES)), trace=_trace)

    outf = np.empty((B, T, N), dtype=np.float32)
    for c in range(NCORES):
        b, j, jb = _core_chunks(c)
        o = res.results[c]["out"]
        outf[b, j * CH:(j + 1) * CH] = o[:, :CH].T
        outf[b, jb * CH:(jb + 1) * CH] = o[:, CH:].T
    if _trace:
        kernel.last_results = res
    return outf


# revision 58
# speedup vs baseline: 1.2571x; 1.0698x over previous
"""Trainium2 Bass kernel for a causal AttentionBlock (dense transformer).

Model (reference):
    qkv = x @ Wqkv + bqkv ; 16-head causal attention (no out-proj)
    x2  = x + attn_out
    out = x2 + relu(x2 @ W1 + b1) @ W2 + b2

x: [2, 2048, 1024] fp32. 8 NeuronCores.

Sharding (no collectives): data-parallel over (batch, query-chunk). Core c
handles batch b = c//4 and the balanced causal chunk pair (j, 7-j), j = c%4,
of 8x256-row chunks, giving every core the same 512 query rows. Each core
redundantly projects K/V for its whole batch (uniform SPMD program), computes
attention for its rows, then the MLP for its rows. Host concatenates.

All matmul operands are bf16 (fp32 PSUM accumulate); fp32r triggers HAM power
throttling (util limit 0.5 for >50% of the kernel) and costs 1.5 cyc/row on
LDWEIGHTS. The attention residual add runs in fp32 (xqb operand); x2 is then
kept in bf16 (it only feeds bf16 matmuls and the final residual add).

Everything on-chip runs transposed ([feature, row] layout); x arrives
pre-transposed from the host and the output returns transposed, so the kernel
does zero PE transposes. K stays resident in SBUF (no DRAM spill).

Attention processes 8 kv slots per (head, core): the kv row-blocks are
host-permuted so slots 0-3 serve chunk A and B together (N=512 streams,
slot 3 = A's diagonal), slots 4-7 serve B alone (N=256, slot 7 = B's
diagonal). Per-slot 0/-1e9 gates (per-core data) mask disallowed blocks via
the Exp activation bias; diagonal slots add a constant triangular mask.
Softmax denominators come free via a ones-column appended to V; out-of-range
exp is impossible (scores are bounded) so max-subtraction is skipped.
"""
import os
import sys

sys.path.insert(0, "/opt/trn_rl_repo")

import numpy as np
import ml_dtypes

import bass_rust
import concourse.bass as bass
import concourse.mybir as mybir
import concourse.tile as tile
from concourse.bass_utils import run_bass_kernel_spmd

# ---------------------------------------------------------------- constants
B, T, N = 2, 2048, 1024
H, D = 16, 64
NCORES = 8
CH = 256               # query chunk rows
F32 = mybir.dt.float32
BF16 = mybir.dt.bfloat16
FP8 = mybir.dt.float8e4
NPBF = ml_dtypes.bfloat16
NPF8 = ml_dtypes.float8_e4m3
WS = 64.0              # fp8 weight pre-scale (avoids e4m3 denormals)
DR = mybir.MatmulPerfMode.DoubleRow

_prog_cache = {}


# ------------------------------------------------------------- wait legalizer
def _legalize_waits(nc):
    """This walrus build accepts <=1 sync wait on most instructions and 0 on
    fp32/fp32r Matmult (fused self-loading LDW). Move excess waits onto bare
    EventSemaphore instructions inserted before, on the same engine."""
    n_split = 0
    for fn in nc.m.functions:
        for blk in fn.blocks:
            insts = blk.instructions
            out = []
            for inst in insts:
                si = inst.sync_info
                waits = list(si.on_wait) if si is not None else []
                tname = type(inst).__name__
                if tname in ("InstMatmult", "InstMatmultMx"):
                    maxw = 0
                    for arg in inst.ins:
                        dt = getattr(arg, "dtype", None)
                        if dt is not None and mybir.dt.size(dt) == 2:
                            maxw = 1
                            break
                else:
                    maxw = 1
                if len(waits) > maxw:
                    extra = waits[:-maxw] if maxw else waits
                    keep = waits[-maxw:] if maxw else []
                    for k, w in enumerate(extra):
                        ev = mybir.InstEventSemaphore(
                            name=f"{inst.name}-lw{k}", ins=[], outs=[]
                        )
                        ev.engine = inst.engine
                        ev.sync_info = bass_rust.SyncInfo(on_wait=[w], on_update=[])
                        out.append(ev)
                        n_split += 1
                    inst.sync_info = bass_rust.SyncInfo(
                        on_wait=keep, on_update=list(si.on_update)
                    )
                out.append(inst)
            insts[:] = out
    return n_split


# ------------------------------------------------------------------- program
def _build_program():
    nc = bass.Bass("TRN2", debug=False, num_devices=NCORES)

    t_ = {}
    t_["xqt16"] = nc.dram_tensor("xqt16", [N, 2 * CH], BF16,
                                 kind="ExternalInput").ap()
    t_["xb8"] = nc.dram_tensor("xb8", [4, 128, 2, T], FP8,
                               kind="ExternalInput").ap()
    t_["wq"] = nc.dram_tensor("wq", [8, 128, N], BF16,
                              kind="ExternalInput").ap()
    t_["wk8"] = nc.dram_tensor("wk8", [4, 128, 2, N], FP8,
                               kind="ExternalInput").ap()
    t_["wv8"] = nc.dram_tensor("wv8", [4, 128, 2, N], FP8,
                               kind="ExternalInput").ap()
    t_["w1h"] = nc.dram_tensor("w1h", [32, 128, 8, 128], BF16,
                               kind="ExternalInput").ap()
    t_["w2h"] = nc.dram_tensor("w2h", [8, 128, 32, 128], BF16,
                               kind="ExternalInput").ap()
    for nm, sz in (("bqs", N), ("bk", N), ("bv", N), ("b1", 4 * N),
                   ("b2", N)):
        t_[nm] = nc.dram_tensor(nm, [sz], F32, kind="ExternalInput").ap()
    t_["gta"] = nc.dram_tensor("gta", [128, 3], F32, kind="ExternalInput").ap()
    t_["gtb"] = nc.dram_tensor("gtb", [128, 3], F32, kind="ExternalInput").ap()
    t_["md3"] = nc.dram_tensor("md3", [2, 128, 2 * CH], F32,
                               kind="ExternalInput").ap()
    t_["md7"] = nc.dram_tensor("md7", [128, 2 * CH], F32,
                               kind="ExternalInput").ap()
    t_["out"] = nc.dram_tensor("out", [N, 2 * CH], F32,
                               kind="ExternalOutput").ap()

    with tile.TileContext(nc) as tc:
        _emit(nc, tc, t_)
    return nc


def _emit(nc, tc, t_):
    AF = mybir.ActivationFunctionType
    OP = mybir.AluOpType

    with tc.tile_pool(name="const", bufs=1) as const:
        ones = const.tile([128, D], BF16)
        nc.vector.memset(ones[:], 1.0)
        bias = {}
        const_dmas = []
        for nm, w in (("bqs", 8), ("bk", 8), ("bv", 8), ("b1", 32),
                      ("b2", 8)):
            bias[nm] = const.tile([128, w], F32, name=f"b_{nm}")
            const_dmas.append((bias[nm][:],
                               t_[nm].rearrange("(f p) -> p f", p=128)))
        gta = const.tile([128, 3], F32, name="gta")
        const_dmas.append((gta[:], t_["gta"]))
        gtb = const.tile([128, 3], F32, name="gtb")
        const_dmas.append((gtb[:], t_["gtb"]))
        md3 = const.tile([128, 2, 2 * CH], F32, name="md3")
        const_dmas.append((md3[:], t_["md3"].rearrange("s p q -> p s q")))
        md7 = const.tile([128, 2 * CH], F32, name="md7")
        const_dmas.append((md7[:], t_["md7"]))

        with tc.tile_pool(name="x2t", bufs=8) as px2t:
            x2T = [px2t.tile([128, 2 * CH], BF16, tag="x2t", name=f"x2T{i}")
                   for i in range(8)]
            with tc.tile_pool(name="keep", bufs=1) as keep:
                # v_aug[rt]: [128 kv-rows, head h, [v | 1]]
                v_aug = [keep.tile([128, H, D + 1], BF16, tag=f"va{rt}",
                                   name=f"va{rt}") for rt in range(T // 128)]
                kth = [keep.tile([128, T], BF16, tag=f"kth{f}",
                                 name=f"kth{f}") for f in range(8)]
                qT = [keep.tile([128, 2 * CH], BF16, tag=f"qt{f}",
                                name=f"qT{f}") for f in range(8)]
                # xqb[f] = xq.T + bv (attn-out bias folded into the residual)
                xqb = [keep.tile([128, 2 * CH], BF16, tag=f"xqb{f}",
                                 name=f"xqb{f}") for f in range(8)]

                w1p = _fused_proj_attn(nc, tc, AF, OP, t_, bias, ones, gta,
                                       gtb, md3, md7, v_aug, kth, qT, xqb,
                                       x2T, px2t, const_dmas)
            _phase3(nc, tc, AF, OP, t_, bias, x2T, t_["out"], w1p)


def _fused_proj_attn(nc, tc, AF, OP, t_, bias, ones, gta, gtb, md3, md7,
                     v_aug, kth, qT, xqb, x2T, px2t, const_dmas):
    """Projections + attention, fused so the PE never idles (and never drops
    out of its 2.4GHz p-state): Q and V project densely up front; the
    K-projection of head f+1 is fed as filler matmuls into head f's
    latency-bound attention pipeline.

    Attention per head runs the transposed flow and writes x2T = xq + bv +
    attn (fp32). kv slots 0-3 stream chunk A and B together (N=512, slot 3 =
    A diag with constant tri mask on the A half); slots 4-7 stream B alone
    (N=256, slot 7 = B diag). Per-core gates: multiplicative {0,1} on the A
    half post-exp (DVE), additive {0,-1e9} exp bias for B. AV matmuls and
    the normalize/residual chain are software-pipelined one step behind so
    the in-order PE queue never stalls on DVE/ACT latency."""
    xqt16, xb8d = t_["xqt16"], t_["xb8"]
    wqd, wk8d, wv8d = t_["wq"], t_["wk8"], t_["wv8"]
    with tc.tile_pool(name="fw", bufs=1) as fw, \
         tc.tile_pool(name="fx", bufs=1) as fx, \
         tc.tile_pool(name="ps2s", bufs=4, space="PSUM") as ps2s, \
         tc.tile_pool(name="ps2o", bufs=2, space="PSUM") as ps2o, \
         tc.tile_pool(name="psk", bufs=2, space="PSUM") as psk:

        # ones column of v_aug (all heads: [v | 1])
        for rt in range(T // 128):
            nc.vector.memset(v_aug[rt][:, :, D:D + 1], 1.0)

        wk8 = [fw.tile([128, 2, N], FP8, tag=f"wk8{i}", name=f"wk8{i}")
               for i in range(4)]
        xb8 = [fx.tile([128, 2, T], FP8, tag=f"xb8{i}", name=f"xb8{i}")
               for i in range(4)]

        pend = [None]

        def defer(fn):
            if pend[0] is not None:
                pend[0]()
            pend[0] = fn

        with tc.tile_pool(name="fv", bufs=1) as fv:
            wv8 = [fv.tile([128, 2, N], FP8, tag=f"wv8{i}", name=f"wv8{i}")
                   for i in range(4)]
            with tc.tile_pool(name="fq", bufs=1) as fq:
                wtq = [fq.tile([128, N], BF16, tag=f"wtq{kc}",
                               name=f"wtq{kc}") for kc in range(8)]
                xqm = [fq.tile([128, 2 * CH], BF16, tag=f"xqm{f}",
                               name=f"xqm{f}") for f in range(8)]
                # first-use tensors split across idle queues for parallel DMA
                for kc in range(8):
                    q = (nc.sync, nc.gpsimd)[kc % 2]
                    q.dma_start(wtq[kc][:], wqd[kc, :, :])
                for f in range(8):
                    nc.scalar.dma_start(xqm[f][:],
                                        xqt16[f * 128:(f + 1) * 128, :])
                for i in range(4):
                    nc.sync.dma_start(wv8[i][:], wv8d[i, :, :, :])
                for i in range(4):
                    nc.sync.dma_start(xb8[i][:], xb8d[i, :, :, :])
                for i in range(4):
                    nc.sync.dma_start(wk8[i][:], wk8d[i, :, :, :])
                # constants (biases/gates/masks): first use is ~30us in, so
                # they queue behind the bandwidth-critical front tensors
                for dst, src_ap in const_dmas:
                    nc.gpsimd.dma_start(dst, src_ap)

                # --- Q projection: qT[f] = (Wq.T @ xq.T) * 0.125 + bqs
                for f in range(8):
                    pp = psk.tile([128, 2 * CH], F32, tag="kproj")
                    for kc in range(8):
                        nc.tensor.matmul(
                            pp[:], wtq[kc][:, f * 128:(f + 1) * 128],
                            xqm[kc][:], start=(kc == 0), stop=(kc == 7))
                    defer(lambda pp=pp, f=f: nc.scalar.activation(
                        qT[f][:], pp[:], AF.Identity,
                        bias=bias["bqs"][:, f:f + 1], scale=0.125))
                if pend[0] is not None:
                    pend[0]()
                pend[0] = None
                # residual operand, attn-out bias folded in: xqb = xq.T + bv
                for f in range(8):
                    nc.vector.tensor_scalar_add(xqb[f][:], xqm[f][:],
                                                bias["bv"][:, f:f + 1])

            # --- V projection, dense, fp8 DoubleRow (0.5 cyc/row);
            # weights carry a x64 pre-scale undone at evacuation
            for rb in range(4):
                for rt in range(4):
                    for nb in range(2):
                        pp = psk.tile([128, 512], F32, tag="kproj")
                        for i in range(4):
                            nc.tensor.matmul(
                                pp[:],
                                xb8[i][:, :, rb * 512 + rt * 128:
                                       rb * 512 + (rt + 1) * 128],
                                wv8[i][:, :, nb * 512:(nb + 1) * 512],
                                start=(i == 0), stop=(i == 3), perf_mode=DR)
                        defer(lambda pp=pp, rb=rb, rt=rt, nb=nb:
                              nc.vector.tensor_scalar_mul(
                                  v_aug[rb * 4 + rt][:, nb * 8:(nb + 1) * 8,
                                                     0:D],
                                  pp[:].rearrange("p (h d) -> p h d", d=D),
                                  1.0 / WS))
            if pend[0] is not None:
                pend[0]()
            pend[0] = None

        # first W1 tiles: allocated after the q/v weight pools free their
        # space (and outside the keep pool), so their DMAs stream during
        # attention instead of waiting for it to drain at the phase boundary
        w1p = [px2t.tile([128, 8, 128], BF16, tag=f"w1p{i}", bufs=1,
                         name=f"w1p{i}") for i in range(4)]
        for m in range(4):
            nc.sync.dma_start(w1p[m][:], t_["w1h"][m, :, :, :])

        def kproj_fillers(f):
            """One closure per instruction of head f's K projection:
            kth[f][:, rb*512:+512] = (Wk[:, f].T @ xb.T + bk) as bf16."""
            out = []
            for rb in range(4):
                holder = {}
                for i in range(5):
                    def go(rb=rb, i=i, holder=holder, f=f):
                        if i == 0:
                            holder["pp"] = psk.tile([128, 512], F32,
                                                    tag="kproj",
                                                    name=f"kpp{f}_{rb}")
                        if i < 4:
                            nc.tensor.matmul(
                                holder["pp"][:],
                                wk8[i][:, :, f * 128:(f + 1) * 128],
                                xb8[i][:, :, rb * 512:(rb + 1) * 512],
                                start=(i == 0), stop=(i == 3), perf_mode=DR)
                        else:
                            nc.vector.tensor_scalar(
                                out=kth[f][:, rb * 512:(rb + 1) * 512],
                                in0=holder["pp"][:], scalar1=1.0 / WS,
                                scalar2=bias["bk"][:, f:f + 1],
                                op0=mybir.AluOpType.mult,
                                op1=mybir.AluOpType.add)
                    out.append(go)
            return out

        # K projection of head 0, dense; heads 1-7 become attention filler
        for fn in kproj_fillers(0):
            fn()
        fillers = []

        def fill(n=1):
            for _ in range(n):
                if fillers:
                    fillers.pop(0)()

        _attention(nc, tc, AF, OP, bias, ones, gta, gtb, md3, md7, v_aug,
                   kth, qT, xqb, x2T, ps2s, ps2o, fillers,
                   kproj_fillers, fill)
    return w1p


def _attention(nc, tc, AF, OP, bias, ones, gta, gtb, md3, md7, v_aug,
               kth, qT, xqb, x2T, ps2s, ps2o, fillers,
               kproj_fillers, fill):
    with tc.tile_pool(name="p2m", bufs=1) as p2m, \
         tc.tile_pool(name="p2w", bufs=6) as p2w:

        # odd-head residual operands shifted down to partitions 0:64
        xqlo_t = [p2m.tile([128, 2 * CH], BF16, tag=f"xql{i}",
                           name=f"xq_lo{i}") for i in range(8)]

        def xq_lo(f):
            return xqlo_t[f][0:D, :]

        for f in range(8):
            nc.gpsimd.dma_start(xq_lo(f), xqb[f][D:128, :])

        pending = []          # AV work deferred one slot-iteration
        fin_q = []            # finalize work deferred one half-unit

        def flush():
            while len(pending) > 2:
                pending.pop(0)()
            while len(fin_q) > 1:
                fin_q.pop(0)()

        def start_fin(acc):
            # reciprocal of the [1,512] denominator row would use one DVE
            # lane (6.8ns/elem serial). Shuffle it to [128,4] via tiny gpsimd
            # DMAs, reciprocate on 128 lanes, shuffle back: ~10x less DVE
            # occupancy; the latency hides behind the next unit's pipeline.
            dsb = p2w.tile([128, 2 * CH], F32, tag="dsb", bufs=2)
            nc.vector.tensor_copy(dsb[D:D + 1, :], acc[D:D + 1, :])
            dt = p2w.tile([128, 4], F32, tag="dt", bufs=2)
            nc.gpsimd.dma_start(dt[:], dsb[D:D + 1, :])
            rt = p2w.tile([128, 4], F32, tag="rt", bufs=2)
            nc.vector.reciprocal(rt[:], dt[:])
            rec = p2w.tile([128, 2 * CH], F32, tag="rec", bufs=2)
            nc.gpsimd.dma_start(rec[D:D + 1, :], rt[:])
            recb = p2w.tile([128, 2 * CH], BF16, tag="recb", bufs=2)
            nc.vector.tensor_copy(recb[D:D + 1, :], rec[D:D + 1, :])
            return recb

        for f in range(8):
            if f + 1 < 8:
                fillers.extend(kproj_fillers(f + 1))
            x2lo = p2w.tile([128, 2 * CH], BF16, tag="x2lo", name=f"x2lo{f}",
                            bufs=2)
            # last head: odd half (whose fin ends in a gpsimd DMA hop) goes
            # first so the final x2T write is the direct DVE-add path
            for hp in ((1, 0) if f == 7 else (0, 1)):
                h = 2 * f + hp
                po = D * hp
                qh = qT[f][po:po + D, :]
                acc = ps2o.tile([128, 2 * CH], F32, tag="po")

                # slots 0-3: combined [A|B] streams (N=512)
                for slot in range(4):
                    for s in range(2):
                        c = 2 * slot + s
                        ps = ps2s.tile([128, 2 * CH], F32, tag="ps")
                        nc.tensor.matmul(
                            ps[:], kth[f][po:po + D, c * 128:(c + 1) * 128],
                            qh[:], start=True, stop=True)
                        fill(2)
                        ex = p2w.tile([128, 2 * CH], BF16, tag="ex", bufs=8)
                        if slot == 3:
                            sm = p2w.tile([128, 2 * CH], F32, tag="sm",
                                          bufs=2)
                            nc.vector.tensor_tensor(
                                out=sm[:], in0=ps[:], in1=md3[:, s, :],
                                op=OP.add)
                            nc.scalar.activation(ex[:], sm[:], AF.Exp)
                        else:
                            nc.scalar.activation(ex[:], ps[:], AF.Exp)
                            # zero the A half where this slot's block is
                            # causally disallowed (per-core gate in {0,1})
                            nc.vector.tensor_scalar_mul(
                                ex[:, 0:CH], ex[:, 0:CH],
                                gta[:, slot:slot + 1])
                        flush()

                        def mk_av(ex=ex, c=c, h=h, acc=acc):
                            def go():
                                nc.tensor.matmul(
                                    acc[0:D + 1, :], v_aug[c][:, h, :],
                                    ex[:], start=(c == 0), stop=False,
                                    skip_group_check=True)
                            return go
                        pending.append(mk_av())

                while pending:
                    pending.pop(0)()

                # slots 4-7: B-only streams (one ps/exp per slot, 2x N=256)
                for slot in range(4, 8):
                    ps = ps2s.tile([128, 2 * CH], F32, tag="ps")
                    for s in range(2):
                        c = 2 * slot + s
                        nc.tensor.matmul(
                            ps[:, s * CH:(s + 1) * CH],
                            kth[f][po:po + D, c * 128:(c + 1) * 128],
                            qh[:, CH:2 * CH], start=True, stop=True)
                        fill(1)
                    ex = p2w.tile([128, 2 * CH], BF16, tag="ex", bufs=8)
                    if slot == 7:
                        sm = p2w.tile([128, 2 * CH], F32, tag="sm", bufs=2)
                        nc.vector.tensor_tensor(
                            out=sm[:], in0=ps[:], in1=md7[:], op=OP.add)
                        nc.scalar.activation(ex[:], sm[:], AF.Exp)
                    else:
                        nc.scalar.activation(ex[:], ps[:], AF.Exp,
                                             bias=gtb[:, slot - 4:slot - 3])
                    flush()

                    def mk_av(ex=ex, slot=slot, h=h, acc=acc):
                        def go():
                            for s in range(2):
                                c = 2 * slot + s
                                nc.tensor.matmul(
                                    acc[0:D + 1, CH:2 * CH],
                                    v_aug[c][:, h, :],
                                    ex[:, s * CH:(s + 1) * CH],
                                    start=False, stop=(c == 15),
                                    skip_group_check=True)
                        return go
                    pending.append(mk_av())

                while pending:
                    pending.pop(0)()
                recb = start_fin(acc)
                fin_q.append(_mk_fin(nc, AF, OP, p2w, ps2s, ones, recb,
                                     acc, f, hp, xqb, xq_lo, x2T, x2lo))
            # kth[f+1] must be fully emitted before f+1's QKs reference it
            while fillers:
                fillers.pop(0)()
        for fn in fin_q:
            fn()
        fin_q.clear()


def _mk_fin(nc, AF, OP, p2w, ps2s, ones, recb, acc, f, hp,
            xqb, xq_lo, x2T, x2lo):
    def go():
        pb = ps2s.tile([128, 2 * CH], F32, tag="ps", name=f"pb{f}_{hp}")
        nc.tensor.matmul(pb[0:D, :], ones[D:D + 1, :], recb[D:D + 1, :],
                         start=True, stop=True)
        sb = p2w.tile([128, 2 * CH], F32, tag="sb", bufs=2)
        nc.vector.tensor_copy(sb[0:D, :], pb[0:D, :])
        tt = p2w.tile([128, 2 * CH], F32, tag="tt", bufs=2)
        nc.vector.tensor_tensor(out=tt[0:D, :], in0=acc[0:D, :],
                                in1=sb[0:D, :], op=OP.mult)
        if hp == 0:
            nc.vector.tensor_tensor(
                out=x2T[f][0:D, :], in0=tt[0:D, :],
                in1=xqb[f][0:D, :], op=OP.add)
        else:
            nc.vector.tensor_tensor(
                out=x2lo[0:D, :], in0=tt[0:D, :],
                in1=xq_lo(f), op=OP.add)
            nc.gpsimd.dma_start(x2T[f][D:128, :], x2lo[0:D, :])
    return go


def _phase3(nc, tc, AF, OP, t_, bias, x2T, out, w1p):
    """MLP (transposed) + residual; output stays transposed (host undoes)."""
    w1h, w2h = t_["w1h"], t_["w2h"]
    with tc.tile_pool(name="p3h", bufs=8) as p3h, \
         tc.tile_pool(name="p3w1", bufs=4) as p3w1, \
         tc.tile_pool(name="p3w2", bufs=2) as p3w2, \
         tc.tile_pool(name="p3y", bufs=1) as p3y, \
         tc.tile_pool(name="ps3p", bufs=4, space="PSUM") as ps3p:

        hT = [p3h.tile([128, 4, 2 * CH], BF16, tag="ht", name=f"hT{i}")
              for i in range(8)]
        w1ss = {m: w1p[m] for m in range(4)}

        def load_w1(m):
            w1ss[m] = p3w1.tile([128, 8, 128], BF16, tag="w1s", name=f"w1s{m}")
            nc.sync.dma_start(w1ss[m][:], w1h[m, :, :, :])

        load_w1(4)
        load_w1(5)
        ev_pend = [None]
        for m in range(32):
            if 4 <= m + 2 < 32 and (m + 2) not in w1ss:
                load_w1(m + 2)
            w1s = w1ss.pop(m)
            pp = ps3p.tile([128, 2 * CH], F32, tag="proj")
            for kc in range(8):
                nc.tensor.matmul(pp[:], w1s[:, kc, :], x2T[kc][:],
                                 start=(kc == 0), stop=(kc == 7))
            if ev_pend[0] is not None:
                ev_pend[0]()
            ev_pend[0] = (lambda pp=pp, m=m: nc.scalar.activation(
                hT[m // 4][:, m % 4, :], pp[:], AF.Relu,
                bias=bias["b1"][:, m:m + 1]))
        # hT is read by MLP2 below: drain the deferred evac BEFORE emitting it
        ev_pend[0]()
        ev_pend[0] = None

        w2ss = {}

        def load_w2(mo):
            w2ss[mo] = p3w2.tile([128, 32, 128], BF16, tag="w2s", bufs=3,
                                 name=f"w2s{mo}")
            nc.sync.dma_start(w2ss[mo][:], w2h[mo, :, :, :])

        load_w2(0)
        load_w2(1)
        for mo in range(8):
            if mo + 2 < 8:
                load_w2(mo + 2)
            w2s = w2ss.pop(mo)
            pp = ps3p.tile([128, 2 * CH], F32, tag="proj")
            for kc in range(32):
                nc.tensor.matmul(pp[:], w2s[:, kc, :], hT[kc // 4][:, kc % 4, :],
                                 start=(kc == 0), stop=(kc == 31))
            ys = p3y.tile([128, 2 * CH], F32, tag="yt", bufs=3, name=f"ys{mo}")
            nc.scalar.activation(ys[:], pp[:], AF.Identity,
                                 bias=bias["b2"][:, mo:mo + 1])
            nc.vector.tensor_tensor(out=ys[:], in0=ys[:], in1=x2T[mo][:],
                                    op=OP.add)
            (nc.gpsimd, nc.sync, nc.scalar)[mo % 3].dma_start(
                out[mo * 128:(mo + 1) * 128, :], ys[:])


# --------------------------------------------------------------- host driver
def _install_ntff_hook():
    """The container's antenv stub lacks axon_hooks; provide it so
    run_bass_kernel_spmd(trace=True) can capture NTFF profiles via libaxon."""
    import types

    try:
        import antenv.axon_hooks  # noqa: F401
        return
    except ImportError:
        pass
    holder = {"h": None}
    mod = types.ModuleType("antenv.axon_hooks")
    mod.set_axon_ntff_profile_hook = lambda h: holder.__setitem__("h", h)
    mod.get_axon_ntff_profile_hook = lambda: holder["h"]
    sys.modules["antenv.axon_hooks"] = mod
    import antenv

    antenv.axon_hooks = mod
    if "/root/.axon_site" not in sys.path:
        sys.path.insert(0, "/root/.axon_site")
    from trn_agent_boot.trn_boot import _ntff_profile_via_ctypes

    so = "/opt/axon/libaxon_pjrt.so"
    if os.path.exists(so):
        mod.set_axon_ntff_profile_hook(_ntff_profile_via_ctypes(so))


def _get_program():
    key = "v2-bf16"
    if key not in _prog_cache:
        nc = _build_program()
        _legalize_waits(nc)
        _prog_cache[key] = nc
    return _prog_cache[key]


def _prep_shared(Wqkv, W1, W2, bqkv, b1, b2):
    wq = np.ascontiguousarray(
        Wqkv[:, 0:N].reshape(8, 128, N)).astype(NPBF)
    # fp8 K/V weights: [pair i, partition q, t, col] with contraction row
    # (i, q, t); x64 pre-scale keeps e4m3 out of denormals (undone at evac)
    wk8 = np.ascontiguousarray(
        (Wqkv[:, N:2 * N] * WS).reshape(4, 2, 128, N)
        .transpose(0, 2, 1, 3)).astype(NPF8)
    wv8 = np.ascontiguousarray(
        (Wqkv[:, 2 * N:3 * N] * WS).reshape(4, 2, 128, N)
        .transpose(0, 2, 1, 3)).astype(NPF8)
    w1h = np.ascontiguousarray(
        W1.reshape(8, 128, 32, 128).transpose(2, 1, 0, 3)).astype(NPBF)
    w2h = np.ascontiguousarray(
        W2.reshape(32, 128, 8, 128).transpose(2, 1, 0, 3)).astype(NPBF)
    return {
        "wq": wq, "wk8": wk8, "wv8": wv8, "w1h": w1h, "w2h": w2h,
        "bqs": np.ascontiguousarray(bqkv[:N] * 0.125),
        "bk": np.ascontiguousarray(bqkv[N:2 * N]),
        "bv": np.ascontiguousarray(bqkv[2 * N:]),
        "b1": np.ascontiguousarray(b1), "b2": np.ascontiguousarray(b2),
    }


def _core_chunks(c):
    b, j = c // 4, c % 4
    return b, j, 7 - j


def _slot_blocks(j):
    """kv row-block for each of the 8 slots. Slots 0-2: smallest non-diag
    blocks (always allowed for B); slot 3 = A diag (block j); slots 4-6:
    remaining blocks (B-gated); slot 7 = B diag (block 7-j)."""
    rem = sorted(set(range(8)) - {j, 7 - j})
    return rem[:3] + [j] + rem[3:] + [7 - j]


def _make_gates(j):
    order = _slot_blocks(j)
    gta = np.ones((128, 3), np.float32)   # multiplicative, applied post-exp
    gtb = np.zeros((128, 3), np.float32)  # additive, applied pre-exp (bias)
    for i in range(3):
        if order[i] >= j:            # disallowed for A
            gta[:, i] = 0.0
        if order[4 + i] >= 7 - j:    # disallowed for B
            gtb[:, i] = -1e9
    return gta, gtb


def _tri_mask():
    # [s, p, q]: kv row s*128+p vs query q (within the 256-row chunk)
    kv = (np.arange(2)[:, None, None] * 128 + np.arange(128)[None, :, None])
    q = np.arange(CH)[None, None, :]
    return np.where(kv <= q, 0.0, -1e9).astype(np.float32)


_TRI = _tri_mask()                                    # [2, 128, 256]
_MD3 = np.concatenate([_TRI, np.zeros((2, 128, CH), np.float32)],
                      axis=2)                         # [2, 128, 512]
# [128, s*256+q] flat layout for the B-diagonal mask
_MD7 = np.ascontiguousarray(_TRI.transpose(1, 0, 2).reshape(128, 2 * CH))


def kernel(x, Wqkv, bqkv, W1, b1, W2, b2, _trace=False):
    x = np.asarray(x, dtype=np.float32)
    shared = _prep_shared(np.asarray(Wqkv, np.float32),
                          np.asarray(W1, np.float32),
                          np.asarray(W2, np.float32),
                          np.asarray(bqkv, np.float32),
                          np.asarray(b1, np.float32),
                          np.asarray(b2, np.float32))
    in_maps = []
    for c in range(NCORES):
        b, j, jb = _core_chunks(c)
        xq = np.concatenate(
            [x[b, j * CH:(j + 1) * CH], x[b, jb * CH:(jb + 1) * CH]], axis=0)
        xqt = np.ascontiguousarray(xq.T)
        xbp = x[b].reshape(8, CH, N)[_slot_blocks(j)].reshape(T, N)
        xbt = np.ascontiguousarray(xbp.T)
        xb8 = np.ascontiguousarray(
            xbt.reshape(4, 2, 128, T).transpose(0, 2, 1, 3)).astype(NPF8)
        gta, gtb = _make_gates(j)
        in_maps.append({
            **shared,
            "xqt16": xqt.astype(NPBF),
            "xb8": xb8,
            "gta": gta, "gtb": gtb, "md3": _MD3, "md7": _MD7,
        })

    nc = _get_program()
    if _trace:
        _install_ntff_hook()
    res = run_bass_kernel_spmd(nc, in_maps, list(range(NCORES)), trace=_trace)

    outf = np.empty((B, T, N), dtype=np.float32)
    for c in range(NCORES):
        b, j, jb = _core_chunks(c)
        o = res.results[c]["out"]
        outf[b, j * CH:(j + 1) * CH] = o[:, :CH].T
        outf[b, jb * CH:(jb + 1) * CH] = o[:, CH:].T
    if _trace:
        kernel.last_results = res
    return outf


# revision 59
# speedup vs baseline: 1.3081x; 1.0405x over previous
"""Trainium2 Bass kernel for a causal AttentionBlock (dense transformer).

Model (reference):
    qkv = x @ Wqkv + bqkv ; 16-head causal attention (no out-proj)
    x2  = x + attn_out
    out = x2 + relu(x2 @ W1 + b1) @ W2 + b2

x: [2, 2048, 1024] fp32. 8 NeuronCores.

Sharding (no collectives): data-parallel over (batch, query-chunk). Core c
handles batch b = c//4 and the balanced causal chunk pair (j, 7-j), j = c%4,
of 8x256-row chunks, giving every core the same 512 query rows. Each core
redundantly projects K/V for its whole batch (uniform SPMD program), computes
attention for its rows, then the MLP for its rows. Host concatenates.

All matmul operands are bf16 (fp32 PSUM accumulate); fp32r triggers HAM power
throttling (util limit 0.5 for >50% of the kernel) and costs 1.5 cyc/row on
LDWEIGHTS. The attention residual add runs in fp32 (xqb operand); x2 is then
kept in bf16 (it only feeds bf16 matmuls and the final residual add).

Everything on-chip runs transposed ([feature, row] layout); x arrives
pre-transposed from the host and the output returns transposed, so the kernel
does zero PE transposes. K stays resident in SBUF (no DRAM spill).

Attention processes 8 kv slots per (head, core): the kv row-blocks are
host-permuted so slots 0-3 serve chunk A and B together (N=512 streams,
slot 3 = A's diagonal), slots 4-7 serve B alone (N=256, slot 7 = B's
diagonal). Per-slot 0/-1e9 gates (per-core data) mask disallowed blocks via
the Exp activation bias; diagonal slots add a constant triangular mask.
Softmax denominators come free via a ones-column appended to V; out-of-range
exp is impossible (scores are bounded) so max-subtraction is skipped.
"""
import os
import sys

sys.path.insert(0, "/opt/trn_rl_repo")

import numpy as np
import ml_dtypes

import bass_rust
import concourse.bass as bass
import concourse.mybir as mybir
import concourse.tile as tile
from concourse.bass_utils import run_bass_kernel_spmd

# ---------------------------------------------------------------- constants
B, T, N = 2, 2048, 1024
H, D = 16, 64
NCORES = 8
CH = 256               # query chunk rows
F32 = mybir.dt.float32
BF16 = mybir.dt.bfloat16
FP8 = mybir.dt.float8e4
NPBF = ml_dtypes.bfloat16
NPF8 = ml_dtypes.float8_e4m3
WS = 64.0              # fp8 weight pre-scale (avoids e4m3 denormals)
DR = mybir.MatmulPerfMode.DoubleRow

_prog_cache = {}


# ------------------------------------------------------------- wait legalizer
def _legalize_waits(nc):
    """This walrus build accepts <=1 sync wait on most instructions and 0 on
    fp32/fp32r Matmult (fused self-loading LDW). Move excess waits onto bare
    EventSemaphore instructions inserted before, on the same engine."""
    n_split = 0
    for fn in nc.m.functions:
        for blk in fn.blocks:
            insts = blk.instructions
            out = []
            for inst in insts:
                si = inst.sync_info
                waits = list(si.on_wait) if si is not None else []
                tname = type(inst).__name__
                if tname in ("InstMatmult", "InstMatmultMx"):
                    maxw = 0
                    for arg in inst.ins:
                        dt = getattr(arg, "dtype", None)
                        if dt is not None and mybir.dt.size(dt) == 2:
                            maxw = 1
                            break
                else:
                    maxw = 1
                if len(waits) > maxw:
                    extra = waits[:-maxw] if maxw else waits
                    keep = waits[-maxw:] if maxw else []
                    for k, w in enumerate(extra):
                        ev = mybir.InstEventSemaphore(
                            name=f"{inst.name}-lw{k}", ins=[], outs=[]
                        )
                        ev.engine = inst.engine
                        ev.sync_info = bass_rust.SyncInfo(on_wait=[w], on_update=[])
                        out.append(ev)
                        n_split += 1
                    inst.sync_info = bass_rust.SyncInfo(
                        on_wait=keep, on_update=list(si.on_update)
                    )
                out.append(inst)
            insts[:] = out
    return n_split


# ------------------------------------------------------------------- program
def _build_program():
    nc = bass.Bass("TRN2", debug=False, num_devices=NCORES)

    t_ = {}
    t_["xqt16"] = nc.dram_tensor("xqt16", [N, 2 * CH], BF16,
                                 kind="ExternalInput").ap()
    t_["xb8"] = nc.dram_tensor("xb8", [4, 128, 2, T], FP8,
                               kind="ExternalInput").ap()
    t_["wq8"] = nc.dram_tensor("wq8", [4, 128, 2, N], FP8,
                               kind="ExternalInput").ap()
    t_["xq8"] = nc.dram_tensor("xq8", [4, 128, 2, 2 * CH], FP8,
                               kind="ExternalInput").ap()
    t_["wk8"] = nc.dram_tensor("wk8", [4, 128, 2, N], FP8,
                               kind="ExternalInput").ap()
    t_["wv8"] = nc.dram_tensor("wv8", [4, 128, 2, N], FP8,
                               kind="ExternalInput").ap()
    t_["w1h"] = nc.dram_tensor("w1h", [32, 128, 8, 128], BF16,
                               kind="ExternalInput").ap()
    t_["w2h"] = nc.dram_tensor("w2h", [8, 128, 32, 128], BF16,
                               kind="ExternalInput").ap()
    for nm, sz in (("bqs", N), ("bk", N), ("bv", N), ("b1", 4 * N),
                   ("b2", N)):
        t_[nm] = nc.dram_tensor(nm, [sz], F32, kind="ExternalInput").ap()
    t_["gta"] = nc.dram_tensor("gta", [128, 3], F32, kind="ExternalInput").ap()
    t_["gtb"] = nc.dram_tensor("gtb", [128, 3], F32, kind="ExternalInput").ap()
    t_["md3"] = nc.dram_tensor("md3", [2, 128, 2 * CH], F32,
                               kind="ExternalInput").ap()
    t_["md7"] = nc.dram_tensor("md7", [128, 2 * CH], F32,
                               kind="ExternalInput").ap()
    t_["out"] = nc.dram_tensor("out", [N, 2 * CH], F32,
                               kind="ExternalOutput").ap()

    with tile.TileContext(nc) as tc:
        _emit(nc, tc, t_)
    return nc


def _emit(nc, tc, t_):
    AF = mybir.ActivationFunctionType
    OP = mybir.AluOpType

    with tc.tile_pool(name="const", bufs=1) as const:
        ones = const.tile([128, D], BF16)
        nc.vector.memset(ones[:], 1.0)
        bias = {}
        const_dmas = []
        for nm, w in (("bqs", 8), ("bk", 8), ("bv", 8), ("b1", 32),
                      ("b2", 8)):
            bias[nm] = const.tile([128, w], F32, name=f"b_{nm}")
            const_dmas.append((bias[nm][:],
                               t_[nm].rearrange("(f p) -> p f", p=128)))
        gta = const.tile([128, 3], F32, name="gta")
        const_dmas.append((gta[:], t_["gta"]))
        gtb = const.tile([128, 3], F32, name="gtb")
        const_dmas.append((gtb[:], t_["gtb"]))
        md3 = const.tile([128, 2, 2 * CH], F32, name="md3")
        const_dmas.append((md3[:], t_["md3"].rearrange("s p q -> p s q")))
        md7 = const.tile([128, 2 * CH], F32, name="md7")
        const_dmas.append((md7[:], t_["md7"]))

        with tc.tile_pool(name="x2t", bufs=8) as px2t:
            x2T = [px2t.tile([128, 2 * CH], BF16, tag="x2t", name=f"x2T{i}")
                   for i in range(8)]
            with tc.tile_pool(name="keep", bufs=1) as keep:
                # v_aug[rt]: [128 kv-rows, head h, [v | 1]]
                v_aug = [keep.tile([128, H, D + 1], BF16, tag=f"va{rt}",
                                   name=f"va{rt}") for rt in range(T // 128)]
                kth = [keep.tile([128, T], BF16, tag=f"kth{f}",
                                 name=f"kth{f}") for f in range(8)]
                qT = [keep.tile([128, 2 * CH], BF16, tag=f"qt{f}",
                                name=f"qT{f}") for f in range(8)]
                # xqb[f] = xq.T + bv (attn-out bias folded into the residual)
                xqb = [keep.tile([128, 2 * CH], BF16, tag=f"xqb{f}",
                                 name=f"xqb{f}") for f in range(8)]

                w1p = _fused_proj_attn(nc, tc, AF, OP, t_, bias, ones, gta,
                                       gtb, md3, md7, v_aug, kth, qT, xqb,
                                       x2T, px2t, const_dmas)
            _phase3(nc, tc, AF, OP, t_, bias, x2T, t_["out"], w1p)


def _fused_proj_attn(nc, tc, AF, OP, t_, bias, ones, gta, gtb, md3, md7,
                     v_aug, kth, qT, xqb, x2T, px2t, const_dmas):
    """Projections + attention, fused so the PE never idles (and never drops
    out of its 2.4GHz p-state): Q and V project densely up front; the
    K-projection of head f+1 is fed as filler matmuls into head f's
    latency-bound attention pipeline.

    Attention per head runs the transposed flow and writes x2T = xq + bv +
    attn (fp32). kv slots 0-3 stream chunk A and B together (N=512, slot 3 =
    A diag with constant tri mask on the A half); slots 4-7 stream B alone
    (N=256, slot 7 = B diag). Per-core gates: multiplicative {0,1} on the A
    half post-exp (DVE), additive {0,-1e9} exp bias for B. AV matmuls and
    the normalize/residual chain are software-pipelined one step behind so
    the in-order PE queue never stalls on DVE/ACT latency."""
    xb8d = t_["xb8"]
    wq8d, xq8d, wk8d, wv8d = t_["wq8"], t_["xq8"], t_["wk8"], t_["wv8"]
    with tc.tile_pool(name="fw", bufs=1) as fw, \
         tc.tile_pool(name="fx", bufs=1) as fx, \
         tc.tile_pool(name="ps2s", bufs=4, space="PSUM") as ps2s, \
         tc.tile_pool(name="ps2o", bufs=2, space="PSUM") as ps2o, \
         tc.tile_pool(name="psk", bufs=2, space="PSUM") as psk:

        # ones column of v_aug (all heads: [v | 1])
        for rt in range(T // 128):
            nc.vector.memset(v_aug[rt][:, :, D:D + 1], 1.0)

        wk8 = [fw.tile([128, 2, N], FP8, tag=f"wk8{i}", name=f"wk8{i}")
               for i in range(4)]
        xb8 = [fx.tile([128, 2, T], FP8, tag=f"xb8{i}", name=f"xb8{i}")
               for i in range(4)]

        pend = [None]

        def defer(fn):
            if pend[0] is not None:
                pend[0]()
            pend[0] = fn

        with tc.tile_pool(name="fv", bufs=1) as fv:
            wv8 = [fv.tile([128, 2, N], FP8, tag=f"wv8{i}", name=f"wv8{i}")
                   for i in range(4)]
            with tc.tile_pool(name="fq", bufs=1) as fq:
                wq8 = [fq.tile([128, 2, N], FP8, tag=f"wq8{i}",
                               name=f"wq8{i}") for i in range(4)]
                xq8 = [fq.tile([128, 2, 2 * CH], FP8, tag=f"xq8{i}",
                               name=f"xq8{i}") for i in range(4)]
                # first-use tensors split across idle queues for parallel DMA
                for i in range(4):
                    q = (nc.sync, nc.gpsimd)[i % 2]
                    q.dma_start(wq8[i][:], wq8d[i, :, :, :])
                for i in range(4):
                    nc.scalar.dma_start(xq8[i][:], xq8d[i, :, :, :])
                for i in range(4):
                    nc.sync.dma_start(wv8[i][:], wv8d[i, :, :, :])
                for i in range(4):
                    nc.sync.dma_start(xb8[i][:], xb8d[i, :, :, :])
                for i in range(4):
                    nc.sync.dma_start(wk8[i][:], wk8d[i, :, :, :])
                # constants (biases/gates/masks): first use is ~30us in, so
                # they queue behind the bandwidth-critical front tensors
                for dst, src_ap in const_dmas:
                    nc.gpsimd.dma_start(dst, src_ap)

                # --- Q projection (fp8 DoubleRow):
                # qT[f] = (Wq.T @ xq.T) * 0.125 + bqs, /WS undoes pre-scale
                for f in range(8):
                    pp = psk.tile([128, 2 * CH], F32, tag="kproj")
                    for i in range(4):
                        nc.tensor.matmul(
                            pp[:], wq8[i][:, :, f * 128:(f + 1) * 128],
                            xq8[i][:], start=(i == 0), stop=(i == 3),
                            perf_mode=DR)
                    defer(lambda pp=pp, f=f: nc.scalar.activation(
                        qT[f][:], pp[:], AF.Identity,
                        bias=bias["bqs"][:, f:f + 1], scale=0.125 / WS))
                if pend[0] is not None:
                    pend[0]()
                pend[0] = None
                # residual operand, attn-out bias folded in: xqb = xq.T + bv
                xqm = [fq.tile([128, 2 * CH], BF16, tag=f"xqm{f}",
                               name=f"xqm{f}") for f in range(8)]
                for f in range(8):
                    nc.scalar.dma_start(xqm[f][:],
                                        t_["xqt16"][f * 128:(f + 1) * 128, :])
                for f in range(8):
                    nc.vector.tensor_scalar_add(xqb[f][:], xqm[f][:],
                                                bias["bv"][:, f:f + 1])

            # --- V projection, dense, fp8 DoubleRow (0.5 cyc/row);
            # weights carry a x64 pre-scale undone at evacuation
            for rb in range(4):
                for rt in range(4):
                    for nb in range(2):
                        pp = psk.tile([128, 512], F32, tag="kproj")
                        for i in range(4):
                            nc.tensor.matmul(
                                pp[:],
                                xb8[i][:, :, rb * 512 + rt * 128:
                                       rb * 512 + (rt + 1) * 128],
                                wv8[i][:, :, nb * 512:(nb + 1) * 512],
                                start=(i == 0), stop=(i == 3), perf_mode=DR)
                        defer(lambda pp=pp, rb=rb, rt=rt, nb=nb:
                              nc.vector.tensor_scalar_mul(
                                  v_aug[rb * 4 + rt][:, nb * 8:(nb + 1) * 8,
                                                     0:D],
                                  pp[:].rearrange("p (h d) -> p h d", d=D),
                                  1.0 / WS))
            if pend[0] is not None:
                pend[0]()
            pend[0] = None

        # first W1 tiles: allocated after the q/v weight pools free their
        # space (and outside the keep pool), so their DMAs stream during
        # attention instead of waiting for it to drain at the phase boundary
        w1p = [px2t.tile([128, 8, 128], BF16, tag=f"w1p{i}", bufs=1,
                         name=f"w1p{i}") for i in range(4)]
        for m in range(4):
            nc.sync.dma_start(w1p[m][:], t_["w1h"][m, :, :, :])

        def kproj_fillers(f):
            """One closure per instruction of head f's K projection:
            kth[f][:, rb*512:+512] = (Wk[:, f].T @ xb.T + bk) as bf16."""
            out = []
            for rb in range(4):
                holder = {}
                for i in range(5):
                    def go(rb=rb, i=i, holder=holder, f=f):
                        if i == 0:
                            holder["pp"] = psk.tile([128, 512], F32,
                                                    tag="kproj",
                                                    name=f"kpp{f}_{rb}")
                        if i < 4:
                            nc.tensor.matmul(
                                holder["pp"][:],
                                wk8[i][:, :, f * 128:(f + 1) * 128],
                                xb8[i][:, :, rb * 512:(rb + 1) * 512],
                                start=(i == 0), stop=(i == 3), perf_mode=DR)
                        else:
                            nc.vector.tensor_scalar(
                                out=kth[f][:, rb * 512:(rb + 1) * 512],
                                in0=holder["pp"][:], scalar1=1.0 / WS,
                                scalar2=bias["bk"][:, f:f + 1],
                                op0=mybir.AluOpType.mult,
                                op1=mybir.AluOpType.add)
                    out.append(go)
            return out

        # K projection of head 0, dense; heads 1-7 become attention filler
        for fn in kproj_fillers(0):
            fn()
        fillers = []

        def fill(n=1):
            for _ in range(n):
                if fillers:
                    fillers.pop(0)()

        _attention(nc, tc, AF, OP, bias, ones, gta, gtb, md3, md7, v_aug,
                   kth, qT, xqb, x2T, ps2s, ps2o, fillers,
                   kproj_fillers, fill)
    return w1p


def _attention(nc, tc, AF, OP, bias, ones, gta, gtb, md3, md7, v_aug,
               kth, qT, xqb, x2T, ps2s, ps2o, fillers,
               kproj_fillers, fill):
    with tc.tile_pool(name="p2m", bufs=1) as p2m, \
         tc.tile_pool(name="p2w", bufs=6) as p2w:

        # odd-head residual operands shifted down to partitions 0:64
        xqlo_t = [p2m.tile([128, 2 * CH], BF16, tag=f"xql{i}",
                           name=f"xq_lo{i}") for i in range(8)]

        def xq_lo(f):
            return xqlo_t[f][0:D, :]

        for f in range(8):
            nc.gpsimd.dma_start(xq_lo(f), xqb[f][D:128, :])

        pending = []          # AV work deferred one slot-iteration
        fin_q = []            # finalize work deferred one half-unit

        def flush():
            while len(pending) > 2:
                pending.pop(0)()
            while len(fin_q) > 1:
                fin_q.pop(0)()

        def start_fin(acc):
            # reciprocal of the [1,512] denominator row would use one DVE
            # lane (6.8ns/elem serial). Shuffle it to [128,4] via tiny gpsimd
            # DMAs, reciprocate on 128 lanes, shuffle back: ~10x less DVE
            # occupancy; the latency hides behind the next unit's pipeline.
            dsb = p2w.tile([128, 2 * CH], F32, tag="dsb", bufs=2)
            nc.vector.tensor_copy(dsb[D:D + 1, :], acc[D:D + 1, :])
            dt = p2w.tile([128, 4], F32, tag="dt", bufs=2)
            nc.gpsimd.dma_start(dt[:], dsb[D:D + 1, :])
            rt = p2w.tile([128, 4], F32, tag="rt", bufs=2)
            nc.vector.reciprocal(rt[:], dt[:])
            rec = p2w.tile([128, 2 * CH], F32, tag="rec", bufs=2)
            nc.gpsimd.dma_start(rec[D:D + 1, :], rt[:])
            recb = p2w.tile([128, 2 * CH], BF16, tag="recb", bufs=2)
            nc.vector.tensor_copy(recb[D:D + 1, :], rec[D:D + 1, :])
            return recb

        for f in range(8):
            if f + 1 < 8:
                fillers.extend(kproj_fillers(f + 1))
            x2lo = p2w.tile([128, 2 * CH], BF16, tag="x2lo", name=f"x2lo{f}",
                            bufs=2)
            # last head: odd half (whose fin ends in a gpsimd DMA hop) goes
            # first so the final x2T write is the direct DVE-add path
            for hp in ((1, 0) if f == 7 else (0, 1)):
                h = 2 * f + hp
                po = D * hp
                qh = qT[f][po:po + D, :]
                acc = ps2o.tile([128, 2 * CH], F32, tag="po")

                # slots 0-3: combined [A|B] streams (N=512)
                for slot in range(4):
                    for s in range(2):
                        c = 2 * slot + s
                        ps = ps2s.tile([128, 2 * CH], F32, tag="ps")
                        nc.tensor.matmul(
                            ps[:], kth[f][po:po + D, c * 128:(c + 1) * 128],
                            qh[:], start=True, stop=True)
                        fill(2)
                        ex = p2w.tile([128, 2 * CH], BF16, tag="ex", bufs=8)
                        if slot == 3:
                            sm = p2w.tile([128, 2 * CH], F32, tag="sm",
                                          bufs=2)
                            nc.vector.tensor_tensor(
                                out=sm[:], in0=ps[:], in1=md3[:, s, :],
                                op=OP.add)
                            nc.scalar.activation(ex[:], sm[:], AF.Exp)
                        else:
                            nc.scalar.activation(ex[:], ps[:], AF.Exp)
                            # zero the A half where this slot's block is
                            # causally disallowed (per-core gate in {0,1})
                            nc.vector.tensor_scalar_mul(
                                ex[:, 0:CH], ex[:, 0:CH],
                                gta[:, slot:slot + 1])
                        flush()

                        def mk_av(ex=ex, c=c, h=h, acc=acc):
                            def go():
                                nc.tensor.matmul(
                                    acc[0:D + 1, :], v_aug[c][:, h, :],
                                    ex[:], start=(c == 0), stop=False,
                                    skip_group_check=True)
                            return go
                        pending.append(mk_av())

                while pending:
                    pending.pop(0)()

                # slots 4-7: B-only streams (one ps/exp per slot, 2x N=256)
                for slot in range(4, 8):
                    ps = ps2s.tile([128, 2 * CH], F32, tag="ps")
                    for s in range(2):
                        c = 2 * slot + s
                        nc.tensor.matmul(
                            ps[:, s * CH:(s + 1) * CH],
                            kth[f][po:po + D, c * 128:(c + 1) * 128],
                            qh[:, CH:2 * CH], start=True, stop=True)
                        fill(1)
                    ex = p2w.tile([128, 2 * CH], BF16, tag="ex", bufs=8)
                    if slot == 7:
                        sm = p2w.tile([128, 2 * CH], F32, tag="sm", bufs=2)
                        nc.vector.tensor_tensor(
                            out=sm[:], in0=ps[:], in1=md7[:], op=OP.add)
                        nc.scalar.activation(ex[:], sm[:], AF.Exp)
                    else:
                        nc.scalar.activation(ex[:], ps[:], AF.Exp,
                                             bias=gtb[:, slot - 4:slot - 3])
                    flush()

                    def mk_av(ex=ex, slot=slot, h=h, acc=acc):
                        def go():
                            for s in range(2):
                                c = 2 * slot + s
                                nc.tensor.matmul(
                                    acc[0:D + 1, CH:2 * CH],
                                    v_aug[c][:, h, :],
                                    ex[:, s * CH:(s + 1) * CH],
                                    start=False, stop=(c == 15),
                                    skip_group_check=True)
                        return go
                    pending.append(mk_av())

                while pending:
                    pending.pop(0)()
                recb = start_fin(acc)
                fin_q.append(_mk_fin(nc, AF, OP, p2w, ps2s, ones, recb,
                                     acc, f, hp, xqb, xq_lo, x2T, x2lo))
            # kth[f+1] must be fully emitted before f+1's QKs reference it
            while fillers:
                fillers.pop(0)()
        for fn in fin_q:
            fn()
        fin_q.clear()


def _mk_fin(nc, AF, OP, p2w, ps2s, ones, recb, acc, f, hp,
            xqb, xq_lo, x2T, x2lo):
    def go():
        pb = ps2s.tile([128, 2 * CH], F32, tag="ps", name=f"pb{f}_{hp}")
        nc.tensor.matmul(pb[0:D, :], ones[D:D + 1, :], recb[D:D + 1, :],
                         start=True, stop=True)
        sb = p2w.tile([128, 2 * CH], F32, tag="sb", bufs=2)
        nc.vector.tensor_copy(sb[0:D, :], pb[0:D, :])
        tt = p2w.tile([128, 2 * CH], F32, tag="tt", bufs=2)
        nc.vector.tensor_tensor(out=tt[0:D, :], in0=acc[0:D, :],
                                in1=sb[0:D, :], op=OP.mult)
        if hp == 0:
            nc.vector.tensor_tensor(
                out=x2T[f][0:D, :], in0=tt[0:D, :],
                in1=xqb[f][0:D, :], op=OP.add)
        else:
            nc.vector.tensor_tensor(
                out=x2lo[0:D, :], in0=tt[0:D, :],
                in1=xq_lo(f), op=OP.add)
            nc.gpsimd.dma_start(x2T[f][D:128, :], x2lo[0:D, :])
    return go


def _phase3(nc, tc, AF, OP, t_, bias, x2T, out, w1p):
    """MLP (transposed) + residual; output stays transposed (host undoes)."""
    w1h, w2h = t_["w1h"], t_["w2h"]
    with tc.tile_pool(name="p3h", bufs=8) as p3h, \
         tc.tile_pool(name="p3w1", bufs=4) as p3w1, \
         tc.tile_pool(name="p3w2", bufs=2) as p3w2, \
         tc.tile_pool(name="p3y", bufs=1) as p3y, \
         tc.tile_pool(name="ps3p", bufs=4, space="PSUM") as ps3p:

        hT = [p3h.tile([128, 4, 2 * CH], BF16, tag="ht", name=f"hT{i}")
              for i in range(8)]
        w1ss = {m: w1p[m] for m in range(4)}

        def load_w1(m):
            w1ss[m] = p3w1.tile([128, 8, 128], BF16, tag="w1s", name=f"w1s{m}")
            nc.sync.dma_start(w1ss[m][:], w1h[m, :, :, :])

        load_w1(4)
        load_w1(5)
        ev_pend = [None]
        for m in range(32):
            if 4 <= m + 2 < 32 and (m + 2) not in w1ss:
                load_w1(m + 2)
            w1s = w1ss.pop(m)
            pp = ps3p.tile([128, 2 * CH], F32, tag="proj")
            for kc in range(8):
                nc.tensor.matmul(pp[:], w1s[:, kc, :], x2T[kc][:],
                                 start=(kc == 0), stop=(kc == 7))
            if ev_pend[0] is not None:
                ev_pend[0]()
            ev_pend[0] = (lambda pp=pp, m=m: nc.scalar.activation(
                hT[m // 4][:, m % 4, :], pp[:], AF.Relu,
                bias=bias["b1"][:, m:m + 1]))
        # hT is read by MLP2 below: drain the deferred evac BEFORE emitting it
        ev_pend[0]()
        ev_pend[0] = None

        w2ss = {}

        def load_w2(mo):
            w2ss[mo] = p3w2.tile([128, 32, 128], BF16, tag="w2s", bufs=3,
                                 name=f"w2s{mo}")
            nc.sync.dma_start(w2ss[mo][:], w2h[mo, :, :, :])

        load_w2(0)
        load_w2(1)
        for mo in range(8):
            if mo + 2 < 8:
                load_w2(mo + 2)
            w2s = w2ss.pop(mo)
            pp = ps3p.tile([128, 2 * CH], F32, tag="proj")
            for kc in range(32):
                nc.tensor.matmul(pp[:], w2s[:, kc, :], hT[kc // 4][:, kc % 4, :],
                                 start=(kc == 0), stop=(kc == 31))
            ys = p3y.tile([128, 2 * CH], F32, tag="yt", bufs=3, name=f"ys{mo}")
            nc.scalar.activation(ys[:], pp[:], AF.Identity,
                                 bias=bias["b2"][:, mo:mo + 1])
            nc.vector.tensor_tensor(out=ys[:], in0=ys[:], in1=x2T[mo][:],
                                    op=OP.add)
            (nc.gpsimd, nc.sync, nc.scalar)[mo % 3].dma_start(
                out[mo * 128:(mo + 1) * 128, :], ys[:])


# --------------------------------------------------------------- host driver
def _install_ntff_hook():
    """The container's antenv stub lacks axon_hooks; provide it so
    run_bass_kernel_spmd(trace=True) can capture NTFF profiles via libaxon."""
    import types

    try:
        import antenv.axon_hooks  # noqa: F401
        return
    except ImportError:
        pass
    holder = {"h": None}
    mod = types.ModuleType("antenv.axon_hooks")
    mod.set_axon_ntff_profile_hook = lambda h: holder.__setitem__("h", h)
    mod.get_axon_ntff_profile_hook = lambda: holder["h"]
    sys.modules["antenv.axon_hooks"] = mod
    import antenv

    antenv.axon_hooks = mod
    if "/root/.axon_site" not in sys.path:
        sys.path.insert(0, "/root/.axon_site")
    from trn_agent_boot.trn_boot import _ntff_profile_via_ctypes

    so = "/opt/axon/libaxon_pjrt.so"
    if os.path.exists(so):
        mod.set_axon_ntff_profile_hook(_ntff_profile_via_ctypes(so))


def _get_program():
    key = "v2-bf16"
    if key not in _prog_cache:
        nc = _build_program()
        _legalize_waits(nc)
        _prog_cache[key] = nc
    return _prog_cache[key]


def _prep_shared(Wqkv, W1, W2, bqkv, b1, b2):
    wq8 = np.ascontiguousarray(
        (Wqkv[:, 0:N] * WS).reshape(4, 2, 128, N)
        .transpose(0, 2, 1, 3)).astype(NPF8)
    # fp8 K/V weights: [pair i, partition q, t, col] with contraction row
    # (i, q, t); x64 pre-scale keeps e4m3 out of denormals (undone at evac)
    wk8 = np.ascontiguousarray(
        (Wqkv[:, N:2 * N] * WS).reshape(4, 2, 128, N)
        .transpose(0, 2, 1, 3)).astype(NPF8)
    wv8 = np.ascontiguousarray(
        (Wqkv[:, 2 * N:3 * N] * WS).reshape(4, 2, 128, N)
        .transpose(0, 2, 1, 3)).astype(NPF8)
    w1h = np.ascontiguousarray(
        W1.reshape(8, 128, 32, 128).transpose(2, 1, 0, 3)).astype(NPBF)
    w2h = np.ascontiguousarray(
        W2.reshape(32, 128, 8, 128).transpose(2, 1, 0, 3)).astype(NPBF)
    return {
        "wq8": wq8, "wk8": wk8, "wv8": wv8, "w1h": w1h, "w2h": w2h,
        "bqs": np.ascontiguousarray(bqkv[:N] * 0.125),
        "bk": np.ascontiguousarray(bqkv[N:2 * N]),
        "bv": np.ascontiguousarray(bqkv[2 * N:]),
        "b1": np.ascontiguousarray(b1), "b2": np.ascontiguousarray(b2),
    }


def _core_chunks(c):
    b, j = c // 4, c % 4
    return b, j, 7 - j


def _slot_blocks(j):
    """kv row-block for each of the 8 slots. Slots 0-2: smallest non-diag
    blocks (always allowed for B); slot 3 = A diag (block j); slots 4-6:
    remaining blocks (B-gated); slot 7 = B diag (block 7-j)."""
    rem = sorted(set(range(8)) - {j, 7 - j})
    return rem[:3] + [j] + rem[3:] + [7 - j]


def _make_gates(j):
    order = _slot_blocks(j)
    gta = np.ones((128, 3), np.float32)   # multiplicative, applied post-exp
    gtb = np.zeros((128, 3), np.float32)  # additive, applied pre-exp (bias)
    for i in range(3):
        if order[i] >= j:            # disallowed for A
            gta[:, i] = 0.0
        if order[4 + i] >= 7 - j:    # disallowed for B
            gtb[:, i] = -1e9
    return gta, gtb


def _tri_mask():
    # [s, p, q]: kv row s*128+p vs query q (within the 256-row chunk)
    kv = (np.arange(2)[:, None, None] * 128 + np.arange(128)[None, :, None])
    q = np.arange(CH)[None, None, :]
    return np.where(kv <= q, 0.0, -1e9).astype(np.float32)


_TRI = _tri_mask()                                    # [2, 128, 256]
_MD3 = np.concatenate([_TRI, np.zeros((2, 128, CH), np.float32)],
                      axis=2)                         # [2, 128, 512]
# [128, s*256+q] flat layout for the B-diagonal mask
_MD7 = np.ascontiguousarray(_TRI.transpose(1, 0, 2).reshape(128, 2 * CH))


def kernel(x, Wqkv, bqkv, W1, b1, W2, b2, _trace=False):
    x = np.asarray(x, dtype=np.float32)
    shared = _prep_shared(np.asarray(Wqkv, np.float32),
                          np.asarray(W1, np.float32),
                          np.asarray(W2, np.float32),
                          np.asarray(bqkv, np.float32),
                          np.asarray(b1, np.float32),
                          np.asarray(b2, np.float32))
    in_maps = []
    for c in range(NCORES):
        b, j, jb = _core_chunks(c)
        xq = np.concatenate(
            [x[b, j * CH:(j + 1) * CH], x[b, jb * CH:(jb + 1) * CH]], axis=0)
        xqt = np.ascontiguousarray(xq.T)
        xbp = x[b].reshape(8, CH, N)[_slot_blocks(j)].reshape(T, N)
        xbt = np.ascontiguousarray(xbp.T)
        xb8 = np.ascontiguousarray(
            xbt.reshape(4, 2, 128, T).transpose(0, 2, 1, 3)).astype(NPF8)
        gta, gtb = _make_gates(j)
        xq8 = np.ascontiguousarray(
            xqt.reshape(4, 2, 128, 2 * CH).transpose(0, 2, 1, 3)).astype(NPF8)
        in_maps.append({
            **shared,
            "xqt16": xqt.astype(NPBF),
            "xq8": xq8, "xb8": xb8,
            "gta": gta, "gtb": gtb, "md3": _MD3, "md7": _MD7,
        })

    nc = _get_program()
    if _trace:
        _install_ntff_hook()
    res = run_bass_kernel_spmd(nc, in_maps, list(range(NCORES)), trace=_trace)

    outf = np.empty((B, T, N), dtype=np.float32)
    for c in range(NCORES):
        b, j, jb = _core_chunks(c)
        o = res.results[c]["out"]
        outf[b, j * CH:(j + 1) * CH] = o[:, :CH].T
        outf[b, jb * CH:(jb + 1) * CH] = o[:, CH:].T
    if _trace:
        kernel.last_results = res
    return outf
